# revision 27
# baseline (speedup 1.0000x reference)
"""GCN-VAE encoder (2-layer GCN + reparameterize) on 8 Trainium2 NeuronCores.

Strategy (per the dst-sharding hint):
  - Nodes are sharded across the 8 cores (6250 rows each); edges are
    partitioned by dst and sorted by dst within each core.
  - Layer matmuls (x@W1, h1@[W2|W3]) run on the node shard each core owns.
  - The sparse aggregation out[d] = sum_{(s,d) in E} w * feat[s] is computed
    per 128-dst-node "window": edges are chunked into groups of 128, features
    feat[src] are fetched with dma_gather row gathers (two per window — the
    int16 gather index forces a <32768 split of the feature table), and the
    segment-sum is a PE matmul acc += S^T @ G with a one-hot matrix
    S[e, dst_local[e]] = w_e built on the vector engine.
  - Cross-core exchange of the full feature tables (support1, support23)
    happens through host round-trips between three SPMD launches (no
    on-device collectives needed):
      L1: support1_shard = x_shard @ W1
      L2: h1 = relu(spmm(support1_full));  support23_shard = h1 @ [W2|W3]
      L3: [mu|logvar] = relu(spmm(support23_full)); z = eps*exp(logvar)+mu
"""

import sys

for _p in ("/opt/trn_rl_repo", "/root/.axon_site/_ro/trn_rl_repo"):
    if _p not in sys.path:
        sys.path.append(_p)

import numpy as np
import ml_dtypes

import concourse.mybir as mybir
import concourse.tile as tile
from concourse import bacc
from concourse.bass_utils import run_bass_kernel_spmd
from concourse.masks import make_identity

# ---- problem constants (hardcoded per harness contract) ----
N, E, F_IN, H1, H2 = 50000, 1600000, 512, 256, 64
H23 = 2 * H2                      # concat(mu, logvar) feature width
M = 8                             # cores
NSH = N // M                      # nodes per core
P = 128                           # partitions / window size / edge chunk
NWIN = (NSH + P - 1) // P         # dst windows per core (49)
KCH = F_IN // P                   # k-chunks for layer-1 matmul (4)
SPLIT = 32768                     # int16 gather-index limit

f32 = mybir.dt.float32
f16 = mybir.dt.float16
i16 = mybir.dt.int16
u32 = mybir.dt.uint32

# L3 hybrid-lane parameters: edges of the first AP_SLOTS dst-slots whose src
# is below APN are gathered on-chip (gpsimd ap_gather from an SBUF-resident
# u32 node-pair-packed transposed sup23 table) instead of via DMA descriptors;
# this splits the gather load across the Pool engine and the DMA engines.
APN = 28672                       # nodes covered by the ap-lane pair table
APE = APN // 2                    # pair elements (u32 cols)
APQ = APE // 2                    # gather-view half (table-term balance)
AP_SLOTS = 36                     # slots using the ap-lane (rest pure DMA)
GRP = 6                           # ap slots per gather group

DT = {"f32": mybir.dt.float32, "f32r": mybir.dt.float32r,
      "f16": mybir.dt.float16, "bf16": mybir.dt.bfloat16}
NPDT = {"f32": np.float32, "f32r": np.float32,
        "f16": np.float16, "bf16": ml_dtypes.bfloat16}

# dtype ladder (accuracy/speed): "f32" exact, "f32r" single-pass PE fp32,
# "f16"/"bf16" half-width gather tables.
L1_DT = "f16"                     # x/W1 matmul operand dtype
SUP1_DT = "f16"                  # layer-1 feature table + S dtype
SUP23_DT = "f16"                  # layer-2/3 feature table + S dtype

_PROG_CACHE: dict = {}


# ---------------------------------------------------------------- host prep
def _wrap16(arr):
    """int16 gather-index layout: ordinal i -> [i%16, i//16], replicated to
    128 partitions (8 Q7 cores x 16)."""
    w = arr.astype(np.int16).reshape(-1, 16).T
    return np.tile(w, (8, 1))


def _prep_edges(edge_src, edge_dst, edge_weight):
    """Partition edges by dst shard, sort by (window, src-half), lay out
    per-window chunk tiles.

    Device arrays per core:
      EIDX [128, (totA+totB)*8] int16 — per window [A idxs | B idxs] wrapped
      EMETA [128, 2*totc] f32 — per window [dst_local (nw) | weight (nw)]
    Returns (key, meta, shards); meta holds the static chunk structure
    (identical across cores by construction)."""
    edge_src = np.asarray(edge_src).astype(np.int64)
    edge_dst = np.asarray(edge_dst).astype(np.int64)
    edge_weight = np.asarray(edge_weight).astype(np.float32)

    percore = []
    cntA = np.zeros((M, NWIN), np.int64)
    cntB = np.zeros((M, NWIN), np.int64)
    cntT = np.zeros((M, NWIN), np.int64)
    perms = []
    for m in range(M):
        sel = (edge_dst >= m * NSH) & (edge_dst < (m + 1) * NSH)
        d = edge_dst[sel] - m * NSH
        s = edge_src[sel]
        w = edge_weight[sel]
        win0 = d >> 7
        # slot-balance: rank windows by edge count so slot j holds each
        # core's j-th-largest window -> cross-core max padding shrinks
        wcnt = np.bincount(win0, minlength=NWIN)
        perm = np.argsort(-wcnt, kind="stable")           # slot -> window
        inv = np.empty(NWIN, np.int64)
        inv[perm] = np.arange(NWIN)                       # window -> slot
        perms.append(perm)
        win = inv[win0]                                   # slot index per edge
        klass = np.where(s < N - SPLIT, 0,
                         np.where(s < SPLIT, 1, 2))       # A-only/flex/B-only
        order = np.lexsort((klass, win))
        d, s, w, win, klass = d[order], s[order], w[order], win[order], klass[order]
        dloc = d - (perm[win] << 7)                       # dst_local in window
        for h, cnt in ((0, cntA), (2, cntB)):
            msk = klass == h
            cnt[m] = np.bincount(win[msk], minlength=NWIN)
        cntT[m] = np.bincount(win, minlength=NWIN)
        percore.append((dloc, s, w, win, klass))

    # chunk budget per slot: total rounded up, A sized to forced-A max,
    # flex edges fill A chunks to capacity before B
    tots = np.stack([np.bincount(pc[3], minlength=NWIN) for pc in percore])
    ncwT = np.maximum(2, -(-tots.max(axis=0) // P))
    ncwA = np.maximum(1, -(-cntA.max(axis=0) // P))
    while True:
        bad = (cntB.max(axis=0) > (ncwT - ncwA) * P)
        if not bad.any():
            break
        ncwT[bad] += 1
    ncwB = ncwT - ncwA
    ncw = ncwT
    offs = np.concatenate([[0], np.cumsum(ncw)])
    offsA = np.concatenate([[0], np.cumsum(ncwA)])
    offsB = np.concatenate([[0], np.cumsum(ncwB)])
    totc = int(offs[-1])
    totA, totB = int(offsA[-1]), int(offsB[-1])
    # exact (16-rounded) per-slot gather counts: cross-core max of edges
    # actually landing in each A/B half -> num_idxs below chunk-granular pad
    perA = np.minimum(cntT, ncwA[None, :] * P)
    exactA = (-(-perA.max(axis=0) // 16) * 16).astype(np.int64)
    exactB = (-(-(cntT - perA).max(axis=0) // 16) * 16).astype(np.int64)

    shards = []
    for m in range(M):
        dloc, s, w, win, klass = percore[m]
        DSTL = np.zeros((P, totc), np.float32)
        WGT = np.zeros((P, totc), np.float32)
        srcA = np.zeros(totA * P, np.int64)
        srcB = np.zeros(totB * P, np.int64)
        # edges are slot-major, class-ordered (A-only, flex, B-only): the
        # first capA go to half A (flex spills into A until its chunks are
        # full), the rest to half B
        wcnt = np.bincount(win, minlength=NWIN)
        starts = np.concatenate([[0], np.cumsum(wcnt)])[:-1]
        jall = np.arange(len(dloc)) - starts[win]          # rank within slot
        capA = ncwA[win] * P
        nAB = np.minimum(wcnt, ncwA * P)                   # A edges per slot
        toA = jall < capA
        for h, offsH, srcH, base_extra, sub in (
                (True, offsA, srcA, 0, 0),
                (False, offsB, srcB, None, SPLIT)):
            msk = toA if h else ~toA
            dh, sh, wh, winh = dloc[msk], s[msk], w[msk], win[msk]
            j = jall[msk] if h else jall[msk] - nAB[winh]
            rows = j % P
            base = offs[winh] + (0 if h else ncwA[winh])
            cols = base + j // P
            DSTL[rows, cols] = dh
            WGT[rows, cols] = wh
            srcH[offsH[winh] * P + j] = sh - (0 if h else N - SPLIT)
        IDXA = _wrap16(srcA)                               # [128, totA*8]
        IDXB = _wrap16(srcB)
        EIDX = np.zeros((P, (totA + totB) * 8), np.int16)
        EMETA = np.zeros((P, 2 * totc), np.float32)
        for t in range(NWIN):
            nA, nB = int(ncwA[t]), int(ncwB[t])
            co = (int(offsA[t]) + int(offsB[t])) * 8
            EIDX[:, co:co + nA * 8] = IDXA[:, offsA[t] * 8:(offsA[t] + nA) * 8]
            EIDX[:, co + nA * 8:co + (nA + nB) * 8] = \
                IDXB[:, offsB[t] * 8:(offsB[t] + nB) * 8]
            o = int(offs[t])
            EMETA[:, 2 * o:2 * o + (nA + nB)] = DSTL[:, o:o + nA + nB]
            EMETA[:, 2 * o + nA + nB:2 * (o + nA + nB)] = WGT[:, o:o + nA + nB]
        shards.append((EIDX, EMETA, perms[m]))

    key = (tuple(int(v) for v in ncwA) + tuple(int(v) for v in ncwB)
           + tuple(int(v) for v in exactA) + tuple(int(v) for v in exactB))
    meta = (tuple(int(v) for v in ncwA), tuple(int(v) for v in ncwB),
            offs, offsA, offsB, totc, totA, totB,
            tuple(int(v) for v in exactA), tuple(int(v) for v in exactB))
    return key, meta, shards


def _prep_edges3(edge_src, edge_dst, edge_weight):
    """L3 hybrid-lane edge layout.

    Slots 0..AP_SLOTS-1: edges with src < APN go to the ap-lane, laid out per
    (gather-group, table-half) call as 16-rounded (slot, parity) cell blocks
    of pair indices; src >= APN edges stay on the DMA lane (supB gather).
    Slots AP_SLOTS..: plain DMA A/B gathers (no flex balancing).
    Chunk metadata (dst_local | weight) is emitted slot-major into EMETA3."""
    edge_src = np.asarray(edge_src).astype(np.int64)
    edge_dst = np.asarray(edge_dst).astype(np.int64)
    edge_weight = np.asarray(edge_weight).astype(np.float32)

    # per-core per-slot edge lists, same slot permutation as _prep_edges
    cores = []
    for m in range(M):
        sel = (edge_dst >= m * NSH) & (edge_dst < (m + 1) * NSH)
        d = edge_dst[sel] - m * NSH
        s = edge_src[sel]
        w = edge_weight[sel]
        win0 = d >> 7
        wcnt = np.bincount(win0, minlength=NWIN)
        perm = np.argsort(-wcnt, kind="stable")
        inv = np.empty(NWIN, np.int64)
        inv[perm] = np.arange(NWIN)
        win = inv[win0]
        dloc = d - (perm[win] << 7)
        # cell id: ap slots: 0..3 = (half,parity), 4 = hi(DMA); dma slots: 5=A, 6=B
        is_ap = (win < AP_SLOTS) & (s < APN)
        cell = np.where(is_ap, ((s >> 1) >= APQ) * 2 + (s & 1),
                        np.where(win < AP_SLOTS, 4, np.where(s < SPLIT, 5, 6)))
        order = np.lexsort((cell, win))
        cores.append((dloc[order], s[order], w[order], win[order], cell[order]))

    # cross-core max counts per (slot, cell)
    cnt = np.zeros((M, NWIN, 7), np.int64)
    for m, (dloc, s, w, win, cell) in enumerate(cores):
        np.add.at(cnt[m], (win, cell), 1)
    cmax = cnt.max(axis=0)                       # [NWIN, 7]

    r16 = lambda v: -(-int(v) // 16) * 16
    nch_of = lambda v: max(0, -(-int(v) // P))

    groups = [list(range(g * GRP, min((g + 1) * GRP, AP_SLOTS)))
              for g in range(-(-AP_SLOTS // GRP))]
    dma_slots = list(range(AP_SLOTS, NWIN))
    nsp = len(groups)
    dma_after = [dma_slots[(len(dma_slots) * g) // nsp:
                           (len(dma_slots) * (g + 1)) // nsp] for g in range(nsp)]

    # ---- static layout
    apslots = {}
    calls = {}
    ic3 = 0
    for g, grp in enumerate(groups):
        for h in (0, 1):
            pos = 0
            for sl in grp:
                for par in (0, 1):
                    c = 2 * h + par
                    raw = cmax[sl, c]
                    nchc = nch_of(raw)
                    apslots.setdefault(sl, {"cells": [], "hi": None})
                    # blocks are 128-chunk aligned: the PE transpose's strided
                    # lhsT view silently corrupts at non-128 base offsets
                    apslots[sl]["cells"].append((h, par, pos, nchc))
                    pos += P * nchc
            nidx = max(pos, 16)
            calls[(g, h)] = (nidx, ic3)
            ic3 += nidx // 16

    icd = 0
    totc = 0
    for sl in range(NWIN):
        if sl < AP_SLOTS:
            nhi = nch_of(cmax[sl, 4])
            apslots[sl]["hi"] = (nhi, icd, r16(cmax[sl, 4]))
            ncells = sum(nc_ for (_, _, _, nc_) in apslots[sl]["cells"])
            apslots[sl]["ntot"] = ncells + nhi
            apslots[sl]["mc"] = totc
            totc += apslots[sl]["ntot"]
            icd += nhi * 8
        else:
            nA, nB = nch_of(cmax[sl, 5]), nch_of(cmax[sl, 6])
            apslots[sl] = {"dma": (nA, nB, icd, r16(cmax[sl, 5]), r16(cmax[sl, 6])),
                           "mc": totc, "ntot": nA + nB}
            totc += nA + nB
            icd += (nA + nB) * 8
    nidxmax = max(v[0] for v in calls.values())

    m3 = {"groups": groups, "dma_after": dma_after, "calls": calls,
          "apslots": apslots, "ic3": ic3, "icd": icd, "totc": totc,
          "nidxmax": nidxmax}

    # ---- per-core arrays
    shards3 = []
    for m, (dloc, s, w, win, cell) in enumerate(cores):
        EIDX3 = np.zeros((P, ic3), np.int16)
        EIDXD = np.zeros((P, icd), np.int16)
        EMETA3 = np.zeros((P, 2 * totc), np.float32)
        # per (slot, cell) edge slices (cells contiguous per lexsort)
        starts = {}
        k0 = 0
        keyarr = win * 8 + cell
        bounds = np.flatnonzero(np.diff(keyarr)) + 1
        segs = np.split(np.arange(len(win)), bounds)
        for seg in segs:
            if len(seg) == 0:
                continue
            starts[(int(win[seg[0]]), int(cell[seg[0]]))] = seg

        def fill_meta(sl, ci0, nchc, seg):
            ntot = apslots[sl]["ntot"]
            mc = apslots[sl]["mc"]
            if seg is None or nchc == 0:
                return
            j = np.arange(len(seg))
            rows, cols = j % P, ci0 + j // P
            EMETA3[rows, 2 * mc + cols] = dloc[seg]
            EMETA3[rows, 2 * mc + ntot + cols] = w[seg]

        for g, grp in enumerate(groups):
            for h in (0, 1):
                nidx, ioff = calls[(g, h)]
                buf = np.zeros(nidx, np.int64)
                for sl in grp:
                    ci = 0
                    for (hh, par, pos, nchc) in apslots[sl]["cells"]:
                        seg = starts.get((sl, 2 * hh + par))
                        if hh == h and seg is not None:
                            buf[pos:pos + len(seg)] = (s[seg] >> 1) - h * APQ
                        if hh == h:
                            pass
                        ci += nchc
                EIDX3[:, ioff:ioff + nidx // 16] = _wrap16(buf)
        # meta for ap cells + hi, and dma blocks
        for sl in range(NWIN):
            info = apslots[sl]
            if sl < AP_SLOTS:
                ci = 0
                for (hh, par, pos, nchc) in info["cells"]:
                    fill_meta(sl, ci, nchc, starts.get((sl, 2 * hh + par)))
                    ci += nchc
                nhi, doff, cnt16 = info["hi"]
                seg = starts.get((sl, 4))
                fill_meta(sl, ci, nhi, seg)
                if nhi:
                    buf = np.zeros(nhi * P, np.int64)
                    if seg is not None:
                        buf[:len(seg)] = s[seg] - (N - SPLIT)
                    EIDXD[:, doff:doff + nhi * 8] = _wrap16(buf)
            else:
                nA, nB, doff, cA16, cB16 = info["dma"]
                ci = 0
                for cc, nchc, sub, off2 in ((5, nA, 0, 0), (6, nB, N - SPLIT, nA * 8)):
                    seg = starts.get((sl, cc))
                    fill_meta(sl, ci, nchc, seg)
                    if nchc:
                        buf = np.zeros(nchc * P, np.int64)
                        if seg is not None:
                            buf[:len(seg)] = s[seg] - sub
                        EIDXD[:, doff + off2:doff + off2 + nchc * 8] = _wrap16(buf)
                    ci += nchc
        shards3.append((EIDX3, EIDXD, EMETA3))

    key3 = (ic3, icd, totc, nidxmax,
            tuple(apslots[sl]["ntot"] for sl in range(NWIN)),
            tuple(v[0] for v in calls.values()))
    return key3, m3, shards3


# ------------------------------------------------------------- bass builders
def _mk_nc():
    return bacc.Bacc("TRN2", target_bir_lowering=False, debug=False)


def _build_l1():
    """support1_shard[6250,256] = x_shard @ W1.

    xL is host-prepared as [128, NSH, KCH] (xL[p,n,k] = x[n, k*128+p]) so the
    whole shard loads SBUF-resident with four big contiguous DMAs; matmuls
    read stationary tiles straight out of it."""
    dt = DT[L1_DT]
    nc = _mk_nc()
    odt = DT[SUP1_DT]
    xL = nc.dram_tensor("xL", [P, NWIN * P, KCH], dt, kind="ExternalInput")
    W1 = nc.dram_tensor("W1", [F_IN, H1], dt, kind="ExternalInput")
    s1 = nc.dram_tensor("s1", [NWIN * P, H1], odt, kind="ExternalOutput")
    s1r = s1[:].rearrange("(t p) h -> p t h", p=P)          # [128, NWIN, H1]

    NSPLIT = 8
    NPAD = NWIN * P
    spans = [(NPAD * i // NSPLIT, NPAD * (i + 1) // NSPLIT) for i in range(NSPLIT)]
    with tile.TileContext(nc) as tc:
        with tc.tile_pool(name="const", bufs=1) as cpool, \
             tc.tile_pool(name="sbuf", bufs=6) as pool, \
             tc.tile_pool(name="psum", bufs=8, space="PSUM") as psum:
            w1c = cpool.tile([P, KCH, H1], dt)
            nc.sync.dma_start(out=w1c[:],
                              in_=W1[:].rearrange("(k p) n -> p k n", p=P))
            xfull = cpool.tile([P, NWIN * P, KCH], dt)
            for a, b in spans:
                nc.sync.dma_start(out=xfull[:, a:b, :], in_=xL[:, a:b, :])
            ofull = cpool.tile([P, NWIN, H1], odt)
            OSEC = 4
            sec = [(NWIN * i // OSEC, NWIN * (i + 1) // OSEC) for i in range(OSEC)]
            si = 0
            for t in range(NWIN):
                acc = psum.tile([P, H1], f32, space="PSUM", tag="acc")
                for k in range(KCH):
                    nc.tensor.matmul(
                        out=acc[:],
                        lhsT=xfull[:, t * P:(t + 1) * P, k],
                        rhs=w1c[:, k, :],
                        start=(k == 0), stop=(k == KCH - 1))
                nc.scalar.activation(out=ofull[:, t, :], in_=acc[:],
                                     func=mybir.ActivationFunctionType.Copy)
                if t + 1 == sec[si][1]:
                    a, b = sec[si]
                    nc.sync.dma_start(out=s1r[:, a:b, :], in_=ofull[:, a:b, :])
                    si += 1
    nc.compile()
    return nc


def _spmm_windows(nc, pool, psum, gpool, cpool, supA, supB, eidx, emeta,
                  iota_t, meta, H, dt, per_window_out):
    """Shared spmm structure: for each window slot produce PSUM [128, H]
    segment sum, then call per_window_out(win, rows, acc_psum). Edge index
    and metadata arrays are loaded SBUF-resident once up front."""
    ncwA, ncwB, offs, offsA, offsB, totc, totA, totB, exactA, exactB = meta
    metafull = cpool.tile([P, 2 * totc], f32)
    idxfull = cpool.tile([P, (totA + totB) * 8], i16)
    bnds = [0, NWIN // 4, NWIN // 2, (3 * NWIN) // 4, NWIN]
    for a, b in zip(bnds[:-1], bnds[1:]):
        mo, mo2 = 2 * int(offs[a]), 2 * int(offs[b])
        nc.sync.dma_start(out=metafull[:, mo:mo2], in_=emeta[:, mo:mo2])
        io = (int(offsA[a]) + int(offsB[a])) * 8
        io2 = (int(offsA[b]) + int(offsB[b])) * 8
        nc.sync.dma_start(out=idxfull[:, io:io2], in_=eidx[:, io:io2])
    for win in range(NWIN):
        rows = P
        nA, nB = ncwA[win], ncwB[win]
        nw = nA + nB
        off = int(offs[win])
        co = (int(offsA[win]) + int(offsB[win])) * 8
        meta_t = metafull[:, 2 * off:2 * (off + nw)]

        G = gpool.tile([P, nw, H], dt, tag="G")
        # exact counts shave chunk-granular padding; first pool rotation uses
        # the full padded count so no G rows are ever read uninitialized
        cA = nA * P if win < 4 else min(int(exactA[win]), nA * P)
        cB = nB * P if win < 4 else min(int(exactB[win]), nB * P)
        nc.gpsimd.dma_gather(
            out_ap=G[:, 0:nA, :], in_ap=supA[:],
            idxs_ap=idxfull[:, co:co + nA * 8],
            num_idxs=cA, num_idxs_reg=cA, elem_size=H,
            single_packet=False)
        nc.gpsimd.dma_gather(
            out_ap=G[:, nA:nw, :], in_ap=supB[:],
            idxs_ap=idxfull[:, co + nA * 8:co + nw * 8],
            num_idxs=cB, num_idxs_reg=cB, elem_size=H,
            single_packet=False)

        acc = psum.tile([P, H], f32, space="PSUM", tag="acc")
        S = pool.tile([P, nw, P], dt, tag="S")
        for c in range(nw):
            nc.vector.tensor_scalar(
                out=S[:, c, :], in0=iota_t[:],
                scalar1=meta_t[:, c:c + 1], scalar2=meta_t[:, nw + c:nw + c + 1],
                op0=mybir.AluOpType.is_equal, op1=mybir.AluOpType.mult)
            nc.tensor.matmul(
                out=acc[:],
                lhsT=S[:, c, :],
                rhs=G[:, c, :],
                start=(c == 0), stop=(c == nw - 1))
        per_window_out(win, rows, acc)


def _decl_spmm_inputs(nc, meta, H, dt, supname):
    totc, totA, totB = meta[5], meta[6], meta[7]
    supA = nc.dram_tensor(supname + "A", [SPLIT, H], dt, kind="ExternalInput")
    supB = nc.dram_tensor(supname + "B", [SPLIT, H], dt, kind="ExternalInput")
    eidx = nc.dram_tensor("eidx", [P, (totA + totB) * 8], i16, kind="ExternalInput")
    emeta = nc.dram_tensor("emeta", [P, 2 * totc], f32, kind="ExternalInput")
    iota_h = nc.dram_tensor("iota", [P, P], dt, kind="ExternalInput")
    return supA, supB, eidx, emeta, iota_h


def _build_l2(meta):
    """h1 = relu(spmm(support1)); support23_shard = h1 @ W23."""
    dt = DT[SUP1_DT]
    nc = _mk_nc()
    supA, supB, eidx, emeta, iota_h = _decl_spmm_inputs(nc, meta, H1, dt, "sup1")
    W23 = nc.dram_tensor("W23", [H1, H23], f32, kind="ExternalInput")
    odt = DT[SUP23_DT]
    NPAIR = (NWIN + 1) // 2
    s23 = nc.dram_tensor("s23", [NPAIR * P, 2 * H23], odt, kind="ExternalOutput")

    with tile.TileContext(nc) as tc:
        with tc.tile_pool(name="const", bufs=1) as cpool, \
             tc.tile_pool(name="sbuf", bufs=3) as pool, \
             tc.tile_pool(name="small", bufs=8) as spool, \
             tc.tile_pool(name="gpool", bufs=4) as gpool, \
             tc.tile_pool(name="psum", bufs=3, space="PSUM") as psum, \
             tc.tile_pool(name="psum2", bufs=2, space="PSUM") as psum2:
            iota_t = cpool.tile([P, P], dt)
            nc.sync.dma_start(out=iota_t[:], in_=iota_h[:])
            ident = cpool.tile([P, P], f32)
            make_identity(nc, ident[:])
            w23c = cpool.tile([P, H1 // P, H23], f32)
            nc.sync.dma_start(out=w23c[:],
                              in_=W23[:].rearrange("(k p) n -> p k n", p=P))

            opair_box = [None]

            def finish(win, rows, acc):
                h1 = pool.tile([P, H1], f32, tag="h1")
                nc.scalar.activation(out=h1[:], in_=acc[:],
                                     func=mybir.ActivationFunctionType.Relu)
                ps23 = psum2.tile([P, H23], f32, space="PSUM", tag="ps23")
                for fh in range(H1 // P):
                    tp = psum2.tile([P, P], f32, space="PSUM", tag="tp")
                    nc.tensor.transpose(out=tp[:], in_=h1[:, fh * P:(fh + 1) * P],
                                        identity=ident[:])
                    tps = pool.tile([P, P], f32, tag="tps")
                    nc.vector.tensor_copy(out=tps[:], in_=tp[:])
                    nc.tensor.matmul(
                        out=ps23[:],
                        lhsT=tps[:],
                        rhs=w23c[:, fh, :],
                        start=(fh == 0), stop=(fh == H1 // P - 1))
                if win % 2 == 0:
                    op_t = pool.tile([P, 2, H23], odt, tag="opair")
                    opair_box[0] = op_t
                opair = opair_box[0]
                nc.scalar.activation(out=opair[:, win % 2, :], in_=ps23[:],
                                     func=mybir.ActivationFunctionType.Copy)
                pb = win // 2
                if win % 2 == 1:
                    nc.sync.dma_start(out=s23[pb * P:(pb + 1) * P, :],
                                      in_=opair[:])
                elif win == NWIN - 1:
                    nc.sync.dma_start(out=s23[pb * P:(pb + 1) * P, 0:H23],
                                      in_=opair[:, 0, :])

            _spmm_windows(nc, spool, psum, gpool, cpool, supA, supB, eidx,
                          emeta, iota_t, meta, H1, dt, finish)
    nc.compile()
    return nc


def _build_l3(m3):
    """[mu|logvar] = relu(spmm(support23)); z = eps*exp(logvar)+mu.

    Hybrid gather: ap-lane slots pull low-src features from an SBUF-resident
    u32 pair-packed transposed table via gpsimd ap_gather (features land on
    partitions, so each 128-edge chunk is PE-transposed and staged through an
    ACT batch copy before the S matmul); remaining edges use DMA dma_gather."""
    dt = DT[SUP23_DT]
    nc = _mk_nc()
    supA = nc.dram_tensor("sup23A", [SPLIT, H23], dt, kind="ExternalInput")
    supB = nc.dram_tensor("sup23B", [SPLIT, H23], dt, kind="ExternalInput")
    eidx = nc.dram_tensor("eidx", [P, m3["icd"]], i16, kind="ExternalInput")
    eidx3 = nc.dram_tensor("eidx3", [P, m3["ic3"]], i16, kind="ExternalInput")
    emeta = nc.dram_tensor("emeta", [P, 2 * m3["totc"]], f32, kind="ExternalInput")
    t3 = nc.dram_tensor("t3", [P, APE], u32, kind="ExternalInput")
    iota_h = nc.dram_tensor("iota", [P, P], dt, kind="ExternalInput")
    epss = nc.dram_tensor("epss", [P, NWIN * H2], f16, kind="ExternalInput")
    out3 = nc.dram_tensor("out3", [NWIN * P, 3 * H2], f32, kind="ExternalOutput")
    totc = m3["totc"]

    with tile.TileContext(nc) as tc:
        with tc.tile_pool(name="const", bufs=1) as cpool, \
             tc.tile_pool(name="sbuf", bufs=3) as pool, \
             tc.tile_pool(name="spool", bufs=2) as spool, \
             tc.tile_pool(name="gpool", bufs=2) as gpool, \
             tc.tile_pool(name="appool", bufs=2) as appool, \
             tc.tile_pool(name="gspool", bufs=2) as gspool, \
             tc.tile_pool(name="psum", bufs=6, space="PSUM") as psum, \
             tc.tile_pool(name="gtpsum", bufs=2, space="PSUM") as gtp:
            iota_t = cpool.tile([P, P], dt)
            nc.sync.dma_start(out=iota_t[:], in_=iota_h[:])
            ident = cpool.tile([P, P], dt)
            make_identity(nc, ident[:])
            epsfull = cpool.tile([P, NWIN, H2], f16)
            nc.sync.dma_start(out=epsfull[:], in_=epss[:])
            t3sb = cpool.tile([P, APE], u32)
            nc.sync.dma_start(out=t3sb[:, :APE // 2], in_=t3[:, :APE // 2])
            nc.sync.dma_start(out=t3sb[:, APE // 2:], in_=t3[:, APE // 2:])
            metafull = cpool.tile([P, 2 * totc], f32)
            idxfull = cpool.tile([P, m3["icd"]], i16)
            idx3full = cpool.tile([P, m3["ic3"]], i16)
            for t, src, n in ((metafull, emeta, 2 * totc),
                              (idxfull, eidx, m3["icd"]),
                              (idx3full, eidx3, m3["ic3"])):
                for a, b in ((0, n // 2), (n // 2, n)):
                    if b > a:
                        nc.sync.dma_start(out=t[:, a:b], in_=src[:, a:b])

            def finish(win, acc):
                o = pool.tile([P, 3 * H2], f32, tag="o3")
                nc.scalar.activation(out=o[:, H2:H23], in_=acc[:, 0:H2],
                                     func=mybir.ActivationFunctionType.Relu)
                nc.scalar.activation(out=o[:, H23:3 * H2], in_=acc[:, H2:H23],
                                     func=mybir.ActivationFunctionType.Relu)
                ex_t = pool.tile([P, H2], f32, tag="ex")
                nc.scalar.activation(out=ex_t[:], in_=o[:, H23:3 * H2],
                                     func=mybir.ActivationFunctionType.Exp)
                nc.vector.tensor_mul(out=o[:, 0:H2], in0=ex_t[:],
                                     in1=epsfull[:, win, :])
                nc.vector.tensor_add(out=o[:, 0:H2], in0=o[:, 0:H2],
                                     in1=o[:, H2:H23])
                nc.sync.dma_start(out=out3[win * P:(win + 1) * P, :], in_=o[:])

            gcount = [0]

            def mk_S(Sw, meta_t, ntot, c):
                nc.vector.tensor_scalar(
                    out=Sw[:, c, :], in0=iota_t[:],
                    scalar1=meta_t[:, c:c + 1],
                    scalar2=meta_t[:, ntot + c:ntot + c + 1],
                    op0=mybir.AluOpType.is_equal, op1=mybir.AluOpType.mult)

            def process_ap_slot(sl, gviews):
                info = m3["apslots"][sl]
                ntot = info["ntot"]
                mc = info["mc"]
                meta_t = metafull[:, 2 * mc:2 * (mc + ntot)]
                acc = psum.tile([P, H23], f32, space="PSUM", tag="acc")
                Sw = spool.tile([P, ntot, P], dt, tag="S")
                pend = []
                for (h, par, base, nchc) in info["cells"]:
                    for k in range(nchc):
                        pend.append((h, base + P * k, par))
                bi = 0
                while bi < len(pend):
                    bsz = min(8, len(pend) - bi)
                    gt = gtp.tile([P, bsz, P], dt, space="PSUM", tag="GT")
                    for kk in range(bsz):
                        h, q, par = pend[bi + kk]
                        nc.tensor.transpose(out=gt[:, kk, :],
                                            in_=gviews[h][:, q:q + P, par],
                                            identity=ident[:])
                    gs = gspool.tile([P, bsz, P], dt, tag="GS")
                    nc.scalar.activation(out=gs[:], in_=gt[:],
                                         func=mybir.ActivationFunctionType.Copy)
                    for kk in range(bsz):
                        c = bi + kk
                        mk_S(Sw, meta_t, ntot, c)
                        nc.tensor.matmul(out=acc[:], lhsT=Sw[:, c, :],
                                         rhs=gs[:, kk, :],
                                         start=(c == 0), stop=(c == ntot - 1))
                    bi += bsz
                nhi, doff, cnt16 = info["hi"]
                if nhi:
                    G = gpool.tile([P, nhi, H23], dt, tag="G")
                    cx = nhi * P if gcount[0] < 2 else min(cnt16, nhi * P)
                    gcount[0] += 1
                    nc.gpsimd.dma_gather(
                        out_ap=G[:], in_ap=supB[:],
                        idxs_ap=idxfull[:, doff:doff + nhi * 8],
                        num_idxs=cx, num_idxs_reg=cx, elem_size=H23,
                        single_packet=False)
                    for k in range(nhi):
                        c = len(pend) + k
                        mk_S(Sw, meta_t, ntot, c)
                        nc.tensor.matmul(out=acc[:], lhsT=Sw[:, c, :],
                                         rhs=G[:, k, :],
                                         start=(c == 0), stop=(c == ntot - 1))
                finish(sl, acc)

            def process_dma_slot(sl):
                info = m3["apslots"][sl]
                nA, nB, doff, cA16, cB16 = info["dma"]
                ntot = info["ntot"]
                mc = info["mc"]
                meta_t = metafull[:, 2 * mc:2 * (mc + ntot)]
                acc = psum.tile([P, H23], f32, space="PSUM", tag="acc")
                Sw = spool.tile([P, ntot, P], dt, tag="S")
                G = gpool.tile([P, ntot, H23], dt, tag="G")
                for (nch, tab, off2, c16) in ((nA, supA, 0, cA16),
                                              (nB, supB, nA * 8, cB16)):
                    if not nch:
                        continue
                    cx = nch * P if gcount[0] < 2 else min(c16, nch * P)
                    gcount[0] += 1
                    o0 = 0 if off2 == 0 else nA
                    nc.gpsimd.dma_gather(
                        out_ap=G[:, o0:o0 + nch, :], in_ap=tab[:],
                        idxs_ap=idxfull[:, doff + off2:doff + off2 + nch * 8],
                        num_idxs=cx, num_idxs_reg=cx, elem_size=H23,
                        single_packet=False)
                for c in range(ntot):
                    mk_S(Sw, meta_t, ntot, c)
                    nc.tensor.matmul(out=acc[:], lhsT=Sw[:, c, :],
                                     rhs=G[:, c, :],
                                     start=(c == 0), stop=(c == ntot - 1))
                finish(sl, acc)

            for g, grp in enumerate(m3["groups"]):
                gviews = {}
                for h in (0, 1):
                    nidx, ioff = m3["calls"][(g, h)]
                    gapt = appool.tile([P, m3["nidxmax"]], u32, tag="APG")
                    nc.gpsimd.ap_gather(
                        out_ap=gapt[:, 0:nidx], in_ap=t3sb[:, h * APQ:(h + 1) * APQ],
                        idxs_ap=idx3full[:, ioff:ioff + nidx // 16],
                        channels=P, num_elems=APQ, d=1, num_idxs=nidx)
                    gviews[h] = gapt[:].bitcast(dt).rearrange(
                        "p (q two) -> p q two", two=2)
                for sl in grp:
                    process_ap_slot(sl, gviews)
                for sl in m3["dma_after"][g]:
                    process_dma_slot(sl)
    nc.compile()
    return nc


def _get_progs(key, meta, key3, m3):
    ck = (key, key3, L1_DT, SUP1_DT, SUP23_DT)
    if ck not in _PROG_CACHE:
        _PROG_CACHE[ck] = (_build_l1(), _build_l2(meta), _build_l3(m3))
    return _PROG_CACHE[ck]


# ------------------------------------------------------------------- kernel
def _run_spmd(nc, in_maps, tries=4):
    """run_bass_kernel_spmd with retries: the shared device pool occasionally
    needs a few minutes to recover a wedged worker."""
    import time
    for attempt in range(tries):
        try:
            return run_bass_kernel_spmd(nc, in_maps, core_ids=list(range(M)))
        except Exception:
            if attempt == tries - 1:
                raise
            time.sleep(90)


def kernel(x, W1, W2, W3, edge_weight, eps, edge_src, edge_dst):
    x = np.asarray(x, np.float32)
    W1 = np.asarray(W1, np.float32)
    W23 = np.concatenate([np.asarray(W2, np.float32),
                          np.asarray(W3, np.float32)], axis=1)
    eps = np.asarray(eps, np.float32)

    key, meta, eshards = _prep_edges(edge_src, edge_dst, edge_weight)
    key3, m3, eshards3 = _prep_edges3(edge_src, edge_dst, edge_weight)
    nc1, nc2, nc3 = _get_progs(key, meta, key3, m3)

    iota = np.broadcast_to(np.arange(P, dtype=np.float32)[None, :], (P, P))

    # ---- L1: support1 shards
    np1 = NPDT[L1_DT]
    in1 = []
    NPAD = NWIN * P
    for m in range(M):
        xs = np.zeros((NPAD, F_IN), np1)
        xs[:NSH] = x[m * NSH:(m + 1) * NSH].astype(np1)
        xLm = np.ascontiguousarray(
            xs.reshape(NPAD, KCH, P).transpose(2, 0, 1))   # [128, NPAD, KCH]
        in1.append({"xL": xLm, "W1": W1.astype(np1)})
    r1 = _run_spmd(nc1, in1)
    sup1 = np.concatenate([r1.results[m]["s1"][:NSH] for m in range(M)], axis=0)

    # window-slot permutation helpers (slot j on core m = window perm[j])
    def unslot(block, m, H):
        """[NWIN*P, H] slot-blocked -> [NSH, H] node-ordered for core m."""
        perm = eshards[m][2]
        out = np.empty((NSH, H), block.dtype)
        for j in range(NWIN):
            wj = int(perm[j])
            r = min(P, NSH - wj * P)
            out[wj * P:wj * P + r] = block[j * P:j * P + r]
        return out

    def toslot(arr, m):
        """[NSH, H] node-ordered -> [NWIN*P, H] slot-blocked for core m."""
        perm = eshards[m][2]
        out = np.zeros((NWIN * P, arr.shape[1]), arr.dtype)
        for j in range(NWIN):
            wj = int(perm[j])
            r = min(P, NSH - wj * P)
            out[j * P:j * P + r] = arr[wj * P:wj * P + r]
        return out

    # ---- L2: h1 + support23 shards
    np2 = NPDT[SUP1_DT]
    sup1 = sup1.astype(np2)
    in2 = [{"sup1A": sup1[:SPLIT], "sup1B": sup1[N - SPLIT:],
            "eidx": eshards[m][0], "emeta": eshards[m][1],
            "W23": W23, "iota": iota.astype(np2)}
           for m in range(M)]
    r2 = _run_spmd(nc2, in2)
    NPAIR = (NWIN + 1) // 2
    sup23_parts = []
    for m in range(M):
        pr = r2.results[m]["s23"].reshape(NPAIR, P, 2, H23)
        sl = np.empty((NWIN * P, H23), pr.dtype)
        for j in range(NWIN):
            sl[j * P:(j + 1) * P] = pr[j // 2, :, j % 2, :]
        sup23_parts.append(unslot(sl, m, H23))
    sup23 = np.concatenate(sup23_parts, axis=0)

    # ---- L3: mu, logvar, z shards
    np3 = NPDT[SUP23_DT]
    sup23 = sup23.astype(np3)
    t = sup23[:APN].reshape(APE, 2, H23)
    lo = t[:, 0, :].view(np.uint16).astype(np.uint32)
    hi = t[:, 1, :].view(np.uint16).astype(np.uint32)
    T3 = np.ascontiguousarray(((hi << 16) | lo).T)   # [128, APE] u32
    in3 = [{"sup23A": sup23[:SPLIT], "sup23B": sup23[N - SPLIT:],
            "eidx": eshards3[m][1], "eidx3": eshards3[m][0],
            "emeta": eshards3[m][2], "t3": T3,
            "iota": iota.astype(np3),
            "epss": np.ascontiguousarray(
                toslot(eps[m * NSH:(m + 1) * NSH], m)
                .reshape(NWIN, P, H2).transpose(1, 0, 2)
                .reshape(P, NWIN * H2)).astype(np.float16)}
           for m in range(M)]
    r3 = _run_spmd(nc3, in3)
    outs = [unslot(r3.results[m]["out3"], m, 3 * H2) for m in range(M)]
    full = np.concatenate(outs, axis=0)
    z, mu, logvar = full[:, 0:H2], full[:, H2:H23], full[:, H23:3 * H2]
    return (np.ascontiguousarray(z), np.ascontiguousarray(mu),
            np.ascontiguousarray(logvar))



# revision 28
# speedup vs baseline: 1.0084x; 1.0084x over previous
"""GCN-VAE encoder (2-layer GCN + reparameterize) on 8 Trainium2 NeuronCores.

Strategy (per the dst-sharding hint):
  - Nodes are sharded across the 8 cores (6250 rows each); edges are
    partitioned by dst and sorted by dst within each core.
  - Layer matmuls (x@W1, h1@[W2|W3]) run on the node shard each core owns.
  - The sparse aggregation out[d] = sum_{(s,d) in E} w * feat[s] is computed
    per 128-dst-node "window": edges are chunked into groups of 128, features
    feat[src] are fetched with dma_gather row gathers (two per window — the
    int16 gather index forces a <32768 split of the feature table), and the
    segment-sum is a PE matmul acc += S^T @ G with a one-hot matrix
    S[e, dst_local[e]] = w_e built on the vector engine.
  - Cross-core exchange of the full feature tables (support1, support23)
    happens through host round-trips between three SPMD launches (no
    on-device collectives needed):
      L1: support1_shard = x_shard @ W1
      L2: h1 = relu(spmm(support1_full));  support23_shard = h1 @ [W2|W3]
      L3: [mu|logvar] = relu(spmm(support23_full)); z = eps*exp(logvar)+mu
"""

import sys

for _p in ("/opt/trn_rl_repo", "/root/.axon_site/_ro/trn_rl_repo"):
    if _p not in sys.path:
        sys.path.append(_p)

import numpy as np
import ml_dtypes

import concourse.mybir as mybir
import concourse.tile as tile
from concourse import bacc
from concourse.bass_utils import run_bass_kernel_spmd
from concourse.masks import make_identity

# ---- problem constants (hardcoded per harness contract) ----
N, E, F_IN, H1, H2 = 50000, 1600000, 512, 256, 64
H23 = 2 * H2                      # concat(mu, logvar) feature width
M = 8                             # cores
NSH = N // M                      # nodes per core
P = 128                           # partitions / window size / edge chunk
NWIN = (NSH + P - 1) // P         # dst windows per core (49)
KCH = F_IN // P                   # k-chunks for layer-1 matmul (4)
SPLIT = 32768                     # int16 gather-index limit

f32 = mybir.dt.float32
f16 = mybir.dt.float16
i16 = mybir.dt.int16
u32 = mybir.dt.uint32

# L3 hybrid-lane parameters: edges of the first AP_SLOTS dst-slots whose src
# is below APN are gathered on-chip (gpsimd ap_gather from an SBUF-resident
# u32 node-pair-packed transposed sup23 table) instead of via DMA descriptors;
# this splits the gather load across the Pool engine and the DMA engines.
APN = 28672                       # nodes covered by the ap-lane pair table
APE = APN // 2                    # pair elements (u32 cols)
APQ = APE // 2                    # gather-view half (table-term balance)
AP_SLOTS = 36                     # slots using the ap-lane (rest pure DMA)
GRP = 6                           # ap slots per gather group

DT = {"f32": mybir.dt.float32, "f32r": mybir.dt.float32r,
      "f16": mybir.dt.float16, "bf16": mybir.dt.bfloat16}
NPDT = {"f32": np.float32, "f32r": np.float32,
        "f16": np.float16, "bf16": ml_dtypes.bfloat16}

# dtype ladder (accuracy/speed): "f32" exact, "f32r" single-pass PE fp32,
# "f16"/"bf16" half-width gather tables.
L1_DT = "f16"                     # x/W1 matmul operand dtype
SUP1_DT = "f16"                  # layer-1 feature table + S dtype
SUP23_DT = "f16"                  # layer-2/3 feature table + S dtype

_PROG_CACHE: dict = {}


# ---------------------------------------------------------------- host prep
def _wrap16(arr):
    """int16 gather-index layout: ordinal i -> [i%16, i//16], replicated to
    128 partitions (8 Q7 cores x 16)."""
    w = arr.astype(np.int16).reshape(-1, 16).T
    return np.tile(w, (8, 1))


def _prep_edges(edge_src, edge_dst, edge_weight):
    """Partition edges by dst shard, sort by (window, src-half), lay out
    per-window chunk tiles.

    Device arrays per core:
      EIDX [128, (totA+totB)*8] int16 — per window [A idxs | B idxs] wrapped
      EMETA [128, 2*totc] f32 — per window [dst_local (nw) | weight (nw)]
    Returns (key, meta, shards); meta holds the static chunk structure
    (identical across cores by construction)."""
    edge_src = np.asarray(edge_src).astype(np.int64)
    edge_dst = np.asarray(edge_dst).astype(np.int64)
    edge_weight = np.asarray(edge_weight).astype(np.float32)

    percore = []
    cntA = np.zeros((M, NWIN), np.int64)
    cntB = np.zeros((M, NWIN), np.int64)
    cntT = np.zeros((M, NWIN), np.int64)
    perms = []
    for m in range(M):
        sel = (edge_dst >= m * NSH) & (edge_dst < (m + 1) * NSH)
        d = edge_dst[sel] - m * NSH
        s = edge_src[sel]
        w = edge_weight[sel]
        win0 = d >> 7
        # slot-balance: rank windows by edge count so slot j holds each
        # core's j-th-largest window -> cross-core max padding shrinks
        wcnt = np.bincount(win0, minlength=NWIN)
        perm = np.argsort(-wcnt, kind="stable")           # slot -> window
        inv = np.empty(NWIN, np.int64)
        inv[perm] = np.arange(NWIN)                       # window -> slot
        perms.append(perm)
        win = inv[win0]                                   # slot index per edge
        klass = np.where(s < N - SPLIT, 0,
                         np.where(s < SPLIT, 1, 2))       # A-only/flex/B-only
        order = np.lexsort((klass, win))
        d, s, w, win, klass = d[order], s[order], w[order], win[order], klass[order]
        dloc = d - (perm[win] << 7)                       # dst_local in window
        for h, cnt in ((0, cntA), (2, cntB)):
            msk = klass == h
            cnt[m] = np.bincount(win[msk], minlength=NWIN)
        cntT[m] = np.bincount(win, minlength=NWIN)
        percore.append((dloc, s, w, win, klass))

    # chunk budget per slot: total rounded up, A sized to forced-A max,
    # flex edges fill A chunks to capacity before B
    tots = np.stack([np.bincount(pc[3], minlength=NWIN) for pc in percore])
    ncwT = np.maximum(2, -(-tots.max(axis=0) // P))
    ncwA = np.maximum(1, -(-cntA.max(axis=0) // P))
    while True:
        bad = (cntB.max(axis=0) > (ncwT - ncwA) * P)
        if not bad.any():
            break
        ncwT[bad] += 1
    ncwB = ncwT - ncwA
    ncw = ncwT
    offs = np.concatenate([[0], np.cumsum(ncw)])
    offsA = np.concatenate([[0], np.cumsum(ncwA)])
    offsB = np.concatenate([[0], np.cumsum(ncwB)])
    totc = int(offs[-1])
    totA, totB = int(offsA[-1]), int(offsB[-1])
    # exact (16-rounded) per-slot gather counts: cross-core max of edges
    # actually landing in each A/B half -> num_idxs below chunk-granular pad
    perA = np.minimum(cntT, ncwA[None, :] * P)
    exactA = (-(-perA.max(axis=0) // 16) * 16).astype(np.int64)
    exactB = (-(-(cntT - perA).max(axis=0) // 16) * 16).astype(np.int64)

    shards = []
    for m in range(M):
        dloc, s, w, win, klass = percore[m]
        DSTL = np.zeros((P, totc), np.float32)
        WGT = np.zeros((P, totc), np.float32)
        srcA = np.zeros(totA * P, np.int64)
        srcB = np.zeros(totB * P, np.int64)
        # edges are slot-major, class-ordered (A-only, flex, B-only): the
        # first capA go to half A (flex spills into A until its chunks are
        # full), the rest to half B
        wcnt = np.bincount(win, minlength=NWIN)
        starts = np.concatenate([[0], np.cumsum(wcnt)])[:-1]
        jall = np.arange(len(dloc)) - starts[win]          # rank within slot
        capA = ncwA[win] * P
        nAB = np.minimum(wcnt, ncwA * P)                   # A edges per slot
        toA = jall < capA
        for h, offsH, srcH, base_extra, sub in (
                (True, offsA, srcA, 0, 0),
                (False, offsB, srcB, None, SPLIT)):
            msk = toA if h else ~toA
            dh, sh, wh, winh = dloc[msk], s[msk], w[msk], win[msk]
            j = jall[msk] if h else jall[msk] - nAB[winh]
            rows = j % P
            base = offs[winh] + (0 if h else ncwA[winh])
            cols = base + j // P
            DSTL[rows, cols] = dh
            WGT[rows, cols] = wh
            srcH[offsH[winh] * P + j] = sh - (0 if h else N - SPLIT)
        IDXA = _wrap16(srcA)                               # [128, totA*8]
        IDXB = _wrap16(srcB)
        EIDX = np.zeros((P, (totA + totB) * 8), np.int16)
        EMETA = np.zeros((P, 2 * totc), np.float32)
        for t in range(NWIN):
            nA, nB = int(ncwA[t]), int(ncwB[t])
            co = (int(offsA[t]) + int(offsB[t])) * 8
            EIDX[:, co:co + nA * 8] = IDXA[:, offsA[t] * 8:(offsA[t] + nA) * 8]
            EIDX[:, co + nA * 8:co + (nA + nB) * 8] = \
                IDXB[:, offsB[t] * 8:(offsB[t] + nB) * 8]
            o = int(offs[t])
            EMETA[:, 2 * o:2 * o + (nA + nB)] = DSTL[:, o:o + nA + nB]
            EMETA[:, 2 * o + nA + nB:2 * (o + nA + nB)] = WGT[:, o:o + nA + nB]
        shards.append((EIDX, EMETA, perms[m]))

    key = (tuple(int(v) for v in ncwA) + tuple(int(v) for v in ncwB)
           + tuple(int(v) for v in exactA) + tuple(int(v) for v in exactB))
    meta = (tuple(int(v) for v in ncwA), tuple(int(v) for v in ncwB),
            offs, offsA, offsB, totc, totA, totB,
            tuple(int(v) for v in exactA), tuple(int(v) for v in exactB))
    return key, meta, shards


def _prep_edges3(edge_src, edge_dst, edge_weight):
    """L3 hybrid-lane edge layout.

    Slots 0..AP_SLOTS-1: edges with src < APN go to the ap-lane, laid out per
    (gather-group, table-half) call as 16-rounded (slot, parity) cell blocks
    of pair indices; src >= APN edges stay on the DMA lane (supB gather).
    Slots AP_SLOTS..: plain DMA A/B gathers (no flex balancing).
    Chunk metadata (dst_local | weight) is emitted slot-major into EMETA3."""
    edge_src = np.asarray(edge_src).astype(np.int64)
    edge_dst = np.asarray(edge_dst).astype(np.int64)
    edge_weight = np.asarray(edge_weight).astype(np.float32)

    # per-core per-slot edge lists, same slot permutation as _prep_edges
    cores = []
    for m in range(M):
        sel = (edge_dst >= m * NSH) & (edge_dst < (m + 1) * NSH)
        d = edge_dst[sel] - m * NSH
        s = edge_src[sel]
        w = edge_weight[sel]
        win0 = d >> 7
        wcnt = np.bincount(win0, minlength=NWIN)
        perm = np.argsort(-wcnt, kind="stable")
        inv = np.empty(NWIN, np.int64)
        inv[perm] = np.arange(NWIN)
        win = inv[win0]
        dloc = d - (perm[win] << 7)
        # cell id: ap slots: 0..3 = (half,parity), 4 = hi(DMA); dma slots: 5=A, 6=B
        is_ap = (win < AP_SLOTS) & (s < APN)
        cell = np.where(is_ap, ((s >> 1) >= APQ) * 2 + (s & 1),
                        np.where(win < AP_SLOTS, 4, np.where(s < SPLIT, 5, 6)))
        order = np.lexsort((cell, win))
        cores.append((dloc[order], s[order], w[order], win[order], cell[order]))

    # cross-core max counts per (slot, cell)
    cnt = np.zeros((M, NWIN, 7), np.int64)
    for m, (dloc, s, w, win, cell) in enumerate(cores):
        np.add.at(cnt[m], (win, cell), 1)
    cmax = cnt.max(axis=0)                       # [NWIN, 7]

    r16 = lambda v: -(-int(v) // 16) * 16
    nch_of = lambda v: max(0, -(-int(v) // P))

    groups = [list(range(g * GRP, min((g + 1) * GRP, AP_SLOTS)))
              for g in range(-(-AP_SLOTS // GRP))]
    dma_slots = list(range(AP_SLOTS, NWIN))
    nsp = len(groups)
    dma_after = [dma_slots[(len(dma_slots) * g) // nsp:
                           (len(dma_slots) * (g + 1)) // nsp] for g in range(nsp)]

    # ---- static layout
    apslots = {}
    calls = {}
    ic3 = 0
    for g, grp in enumerate(groups):
        for h in (0, 1):
            pos = 0
            for sl in grp:
                for par in (0, 1):
                    c = 2 * h + par
                    raw = cmax[sl, c]
                    nchc = nch_of(raw)
                    apslots.setdefault(sl, {"cells": [], "hi": None})
                    # blocks are 128-chunk aligned: the PE transpose's strided
                    # lhsT view silently corrupts at non-128 base offsets
                    apslots[sl]["cells"].append((h, par, pos, nchc))
                    pos += P * nchc
            nidx = max(pos, 16)
            calls[(g, h)] = (nidx, ic3)
            ic3 += nidx // 16

    icd = 0
    totc = 0
    for sl in range(NWIN):
        if sl < AP_SLOTS:
            nhi = nch_of(cmax[sl, 4])
            apslots[sl]["hi"] = (nhi, icd, r16(cmax[sl, 4]))
            ncells = sum(nc_ for (_, _, _, nc_) in apslots[sl]["cells"])
            apslots[sl]["ntot"] = ncells + nhi
            apslots[sl]["mc"] = totc
            totc += apslots[sl]["ntot"]
            icd += nhi * 8
        else:
            nA, nB = nch_of(cmax[sl, 5]), nch_of(cmax[sl, 6])
            apslots[sl] = {"dma": (nA, nB, icd, r16(cmax[sl, 5]), r16(cmax[sl, 6])),
                           "mc": totc, "ntot": nA + nB}
            totc += nA + nB
            icd += (nA + nB) * 8
    nidxmax = max(v[0] for v in calls.values())

    m3 = {"groups": groups, "dma_after": dma_after, "calls": calls,
          "apslots": apslots, "ic3": ic3, "icd": icd, "totc": totc,
          "nidxmax": nidxmax}

    # ---- per-core arrays
    shards3 = []
    for m, (dloc, s, w, win, cell) in enumerate(cores):
        EIDX3 = np.zeros((P, ic3), np.int16)
        EIDXD = np.zeros((P, icd), np.int16)
        EMETA3 = np.zeros((P, 2 * totc), np.float32)
        # per (slot, cell) edge slices (cells contiguous per lexsort)
        starts = {}
        k0 = 0
        keyarr = win * 8 + cell
        bounds = np.flatnonzero(np.diff(keyarr)) + 1
        segs = np.split(np.arange(len(win)), bounds)
        for seg in segs:
            if len(seg) == 0:
                continue
            starts[(int(win[seg[0]]), int(cell[seg[0]]))] = seg

        def fill_meta(sl, ci0, nchc, seg):
            ntot = apslots[sl]["ntot"]
            mc = apslots[sl]["mc"]
            if seg is None or nchc == 0:
                return
            j = np.arange(len(seg))
            rows, cols = j % P, ci0 + j // P
            EMETA3[rows, 2 * mc + cols] = dloc[seg]
            EMETA3[rows, 2 * mc + ntot + cols] = w[seg]

        for g, grp in enumerate(groups):
            for h in (0, 1):
                nidx, ioff = calls[(g, h)]
                buf = np.zeros(nidx, np.int64)
                for sl in grp:
                    ci = 0
                    for (hh, par, pos, nchc) in apslots[sl]["cells"]:
                        seg = starts.get((sl, 2 * hh + par))
                        if hh == h and seg is not None:
                            buf[pos:pos + len(seg)] = (s[seg] >> 1) - h * APQ
                        if hh == h:
                            pass
                        ci += nchc
                EIDX3[:, ioff:ioff + nidx // 16] = _wrap16(buf)
        # meta for ap cells + hi, and dma blocks
        for sl in range(NWIN):
            info = apslots[sl]
            if sl < AP_SLOTS:
                ci = 0
                for (hh, par, pos, nchc) in info["cells"]:
                    fill_meta(sl, ci, nchc, starts.get((sl, 2 * hh + par)))
                    ci += nchc
                nhi, doff, cnt16 = info["hi"]
                seg = starts.get((sl, 4))
                fill_meta(sl, ci, nhi, seg)
                if nhi:
                    buf = np.zeros(nhi * P, np.int64)
                    if seg is not None:
                        buf[:len(seg)] = s[seg] - (N - SPLIT)
                    EIDXD[:, doff:doff + nhi * 8] = _wrap16(buf)
            else:
                nA, nB, doff, cA16, cB16 = info["dma"]
                ci = 0
                for cc, nchc, sub, off2 in ((5, nA, 0, 0), (6, nB, N - SPLIT, nA * 8)):
                    seg = starts.get((sl, cc))
                    fill_meta(sl, ci, nchc, seg)
                    if nchc:
                        buf = np.zeros(nchc * P, np.int64)
                        if seg is not None:
                            buf[:len(seg)] = s[seg] - sub
                        EIDXD[:, doff + off2:doff + off2 + nchc * 8] = _wrap16(buf)
                    ci += nchc
        shards3.append((EIDX3, EIDXD, EMETA3))

    key3 = (ic3, icd, totc, nidxmax,
            tuple(apslots[sl]["ntot"] for sl in range(NWIN)),
            tuple(v[0] for v in calls.values()))
    return key3, m3, shards3


# ------------------------------------------------------------- bass builders
def _mk_nc():
    return bacc.Bacc("TRN2", target_bir_lowering=False, debug=False)


def _build_l1():
    """support1_shard[6250,256] = x_shard @ W1.

    xL is host-prepared as [128, NSH, KCH] (xL[p,n,k] = x[n, k*128+p]) so the
    whole shard loads SBUF-resident with four big contiguous DMAs; matmuls
    read stationary tiles straight out of it."""
    dt = DT[L1_DT]
    nc = _mk_nc()
    odt = DT[SUP1_DT]
    xL = nc.dram_tensor("xL", [P, NWIN * P, KCH], dt, kind="ExternalInput")
    W1 = nc.dram_tensor("W1", [F_IN, H1], dt, kind="ExternalInput")
    s1 = nc.dram_tensor("s1", [NWIN * P, H1], odt, kind="ExternalOutput")
    s1r = s1[:].rearrange("(t p) h -> p t h", p=P)          # [128, NWIN, H1]

    NSPLIT = 8
    NPAD = NWIN * P
    spans = [(NPAD * i // NSPLIT, NPAD * (i + 1) // NSPLIT) for i in range(NSPLIT)]
    with tile.TileContext(nc) as tc:
        with tc.tile_pool(name="const", bufs=1) as cpool, \
             tc.tile_pool(name="sbuf", bufs=6) as pool, \
             tc.tile_pool(name="psum", bufs=8, space="PSUM") as psum:
            w1c = cpool.tile([P, KCH, H1], dt)
            nc.sync.dma_start(out=w1c[:],
                              in_=W1[:].rearrange("(k p) n -> p k n", p=P))
            xfull = cpool.tile([P, NWIN * P, KCH], dt)
            for a, b in spans:
                nc.sync.dma_start(out=xfull[:, a:b, :], in_=xL[:, a:b, :])
            ofull = cpool.tile([P, NWIN, H1], odt)
            OSEC = 4
            sec = [(NWIN * i // OSEC, NWIN * (i + 1) // OSEC) for i in range(OSEC)]
            si = 0
            for t in range(NWIN):
                acc = psum.tile([P, H1], f32, space="PSUM", tag="acc")
                for k in range(KCH):
                    nc.tensor.matmul(
                        out=acc[:],
                        lhsT=xfull[:, t * P:(t + 1) * P, k],
                        rhs=w1c[:, k, :],
                        start=(k == 0), stop=(k == KCH - 1))
                nc.scalar.activation(out=ofull[:, t, :], in_=acc[:],
                                     func=mybir.ActivationFunctionType.Copy)
                if t + 1 == sec[si][1]:
                    a, b = sec[si]
                    nc.sync.dma_start(out=s1r[:, a:b, :], in_=ofull[:, a:b, :])
                    si += 1
    nc.compile()
    return nc


def _spmm_windows(nc, pool, psum, gpool, cpool, supA, supB, eidx, emeta,
                  iota_t, meta, H, dt, per_window_out):
    """Shared spmm structure: for each window slot produce PSUM [128, H]
    segment sum, then call per_window_out(win, rows, acc_psum). Edge index
    and metadata arrays are loaded SBUF-resident once up front."""
    ncwA, ncwB, offs, offsA, offsB, totc, totA, totB, exactA, exactB = meta
    metafull = cpool.tile([P, 2 * totc], f32)
    idxfull = cpool.tile([P, (totA + totB) * 8], i16)
    bnds = [0, NWIN // 4, NWIN // 2, (3 * NWIN) // 4, NWIN]
    for a, b in zip(bnds[:-1], bnds[1:]):
        mo, mo2 = 2 * int(offs[a]), 2 * int(offs[b])
        nc.sync.dma_start(out=metafull[:, mo:mo2], in_=emeta[:, mo:mo2])
        io = (int(offsA[a]) + int(offsB[a])) * 8
        io2 = (int(offsA[b]) + int(offsB[b])) * 8
        nc.sync.dma_start(out=idxfull[:, io:io2], in_=eidx[:, io:io2])
    for win in range(NWIN):
        rows = P
        nA, nB = ncwA[win], ncwB[win]
        nw = nA + nB
        off = int(offs[win])
        co = (int(offsA[win]) + int(offsB[win])) * 8
        meta_t = metafull[:, 2 * off:2 * (off + nw)]

        G = gpool.tile([P, nw, H], dt, tag="G")
        # exact counts shave chunk-granular padding; first pool rotation uses
        # the full padded count so no G rows are ever read uninitialized
        cA = nA * P if win < 4 else min(int(exactA[win]), nA * P)
        cB = nB * P if win < 4 else min(int(exactB[win]), nB * P)
        nc.gpsimd.dma_gather(
            out_ap=G[:, 0:nA, :], in_ap=supA[:],
            idxs_ap=idxfull[:, co:co + nA * 8],
            num_idxs=cA, num_idxs_reg=cA, elem_size=H,
            single_packet=False)
        nc.gpsimd.dma_gather(
            out_ap=G[:, nA:nw, :], in_ap=supB[:],
            idxs_ap=idxfull[:, co + nA * 8:co + nw * 8],
            num_idxs=cB, num_idxs_reg=cB, elem_size=H,
            single_packet=False)

        acc = psum.tile([P, H], f32, space="PSUM", tag="acc")
        S = pool.tile([P, nw, P], dt, tag="S")
        for c in range(nw):
            nc.vector.tensor_scalar(
                out=S[:, c, :], in0=iota_t[:],
                scalar1=meta_t[:, c:c + 1], scalar2=meta_t[:, nw + c:nw + c + 1],
                op0=mybir.AluOpType.is_equal, op1=mybir.AluOpType.mult)
            nc.tensor.matmul(
                out=acc[:],
                lhsT=S[:, c, :],
                rhs=G[:, c, :],
                start=(c == 0), stop=(c == nw - 1))
        per_window_out(win, rows, acc)


def _decl_spmm_inputs(nc, meta, H, dt, supname):
    totc, totA, totB = meta[5], meta[6], meta[7]
    supA = nc.dram_tensor(supname + "A", [SPLIT, H], dt, kind="ExternalInput")
    supB = nc.dram_tensor(supname + "B", [SPLIT, H], dt, kind="ExternalInput")
    eidx = nc.dram_tensor("eidx", [P, (totA + totB) * 8], i16, kind="ExternalInput")
    emeta = nc.dram_tensor("emeta", [P, 2 * totc], f32, kind="ExternalInput")
    iota_h = nc.dram_tensor("iota", [P, P], dt, kind="ExternalInput")
    return supA, supB, eidx, emeta, iota_h


def _build_l2(meta):
    """h1 = relu(spmm(support1)); support23_shard = h1 @ W23."""
    dt = DT[SUP1_DT]
    nc = _mk_nc()
    supA, supB, eidx, emeta, iota_h = _decl_spmm_inputs(nc, meta, H1, dt, "sup1")
    W23 = nc.dram_tensor("W23", [H1, H23], f32, kind="ExternalInput")
    odt = DT[SUP23_DT]
    NPAIR = (NWIN + 1) // 2
    s23 = nc.dram_tensor("s23", [NPAIR * P, 2 * H23], odt, kind="ExternalOutput")

    with tile.TileContext(nc) as tc:
        with tc.tile_pool(name="const", bufs=1) as cpool, \
             tc.tile_pool(name="sbuf", bufs=3) as pool, \
             tc.tile_pool(name="small", bufs=8) as spool, \
             tc.tile_pool(name="gpool", bufs=4) as gpool, \
             tc.tile_pool(name="psum", bufs=3, space="PSUM") as psum, \
             tc.tile_pool(name="psum2", bufs=2, space="PSUM") as psum2:
            iota_t = cpool.tile([P, P], dt)
            nc.sync.dma_start(out=iota_t[:], in_=iota_h[:])
            ident = cpool.tile([P, P], f32)
            make_identity(nc, ident[:])
            w23c = cpool.tile([P, H1 // P, H23], f32)
            nc.sync.dma_start(out=w23c[:],
                              in_=W23[:].rearrange("(k p) n -> p k n", p=P))

            opair_box = [None]

            def finish(win, rows, acc):
                h1 = pool.tile([P, H1], f32, tag="h1")
                nc.scalar.activation(out=h1[:], in_=acc[:],
                                     func=mybir.ActivationFunctionType.Relu)
                ps23 = psum2.tile([P, H23], f32, space="PSUM", tag="ps23")
                for fh in range(H1 // P):
                    tp = psum2.tile([P, P], f32, space="PSUM", tag="tp")
                    nc.tensor.transpose(out=tp[:], in_=h1[:, fh * P:(fh + 1) * P],
                                        identity=ident[:])
                    tps = pool.tile([P, P], f32, tag="tps")
                    nc.vector.tensor_copy(out=tps[:], in_=tp[:])
                    nc.tensor.matmul(
                        out=ps23[:],
                        lhsT=tps[:],
                        rhs=w23c[:, fh, :],
                        start=(fh == 0), stop=(fh == H1 // P - 1))
                if win % 2 == 0:
                    op_t = pool.tile([P, 2, H23], odt, tag="opair")
                    opair_box[0] = op_t
                opair = opair_box[0]
                nc.scalar.activation(out=opair[:, win % 2, :], in_=ps23[:],
                                     func=mybir.ActivationFunctionType.Copy)
                pb = win // 2
                if win % 2 == 1:
                    nc.sync.dma_start(out=s23[pb * P:(pb + 1) * P, :],
                                      in_=opair[:])
                elif win == NWIN - 1:
                    nc.sync.dma_start(out=s23[pb * P:(pb + 1) * P, 0:H23],
                                      in_=opair[:, 0, :])

            _spmm_windows(nc, spool, psum, gpool, cpool, supA, supB, eidx,
                          emeta, iota_t, meta, H1, dt, finish)
    nc.compile()
    return nc


def _build_l3(m3):
    """[mu|logvar] = relu(spmm(support23)); z = eps*exp(logvar)+mu.

    Hybrid gather: ap-lane slots pull low-src features from an SBUF-resident
    u32 pair-packed transposed table via gpsimd ap_gather (features land on
    partitions, so each 128-edge chunk is PE-transposed and staged through an
    ACT batch copy before the S matmul); remaining edges use DMA dma_gather."""
    dt = DT[SUP23_DT]
    nc = _mk_nc()
    supA = nc.dram_tensor("sup23A", [SPLIT, H23], dt, kind="ExternalInput")
    supB = nc.dram_tensor("sup23B", [SPLIT, H23], dt, kind="ExternalInput")
    eidx = nc.dram_tensor("eidx", [P, m3["icd"]], i16, kind="ExternalInput")
    eidx3 = nc.dram_tensor("eidx3", [P, m3["ic3"]], i16, kind="ExternalInput")
    emeta = nc.dram_tensor("emeta", [P, 2 * m3["totc"]], f32, kind="ExternalInput")
    t3 = nc.dram_tensor("t3", [P, APE], u32, kind="ExternalInput")
    iota_h = nc.dram_tensor("iota", [P, P], dt, kind="ExternalInput")
    epss = nc.dram_tensor("epss", [P, NWIN * H2], f16, kind="ExternalInput")
    out3 = nc.dram_tensor("out3", [NWIN * P, 3 * H2], f32, kind="ExternalOutput")
    totc = m3["totc"]

    with tile.TileContext(nc) as tc:
        with tc.tile_pool(name="const", bufs=1) as cpool, \
             tc.tile_pool(name="sbuf", bufs=3) as pool, \
             tc.tile_pool(name="spool", bufs=2) as spool, \
             tc.tile_pool(name="gpool", bufs=2) as gpool, \
             tc.tile_pool(name="appool", bufs=2) as appool, \
             tc.tile_pool(name="gspool", bufs=2) as gspool, \
             tc.tile_pool(name="psum", bufs=6, space="PSUM") as psum, \
             tc.tile_pool(name="gtpsum", bufs=2, space="PSUM") as gtp:
            iota_t = cpool.tile([P, P], dt)
            nc.sync.dma_start(out=iota_t[:], in_=iota_h[:])
            ident = cpool.tile([P, P], dt)
            make_identity(nc, ident[:])
            epsfull = cpool.tile([P, NWIN, H2], f16)
            nc.sync.dma_start(out=epsfull[:], in_=epss[:])
            t3sb = cpool.tile([P, APE], u32)
            nc.sync.dma_start(out=t3sb[:, :APE // 2], in_=t3[:, :APE // 2])
            nc.sync.dma_start(out=t3sb[:, APE // 2:], in_=t3[:, APE // 2:])
            metafull = cpool.tile([P, 2 * totc], f32)
            idxfull = cpool.tile([P, m3["icd"]], i16)
            idx3full = cpool.tile([P, m3["ic3"]], i16)
            for t, src, n in ((metafull, emeta, 2 * totc),
                              (idxfull, eidx, m3["icd"]),
                              (idx3full, eidx3, m3["ic3"])):
                for a, b in ((0, n // 2), (n // 2, n)):
                    if b > a:
                        nc.sync.dma_start(out=t[:, a:b], in_=src[:, a:b])

            def finish(win, acc):
                o = pool.tile([P, 3 * H2], f32, tag="o3")
                nc.scalar.activation(out=o[:, H2:H23], in_=acc[:, 0:H2],
                                     func=mybir.ActivationFunctionType.Relu)
                nc.scalar.activation(out=o[:, H23:3 * H2], in_=acc[:, H2:H23],
                                     func=mybir.ActivationFunctionType.Relu)
                ex_t = pool.tile([P, H2], f32, tag="ex")
                nc.scalar.activation(out=ex_t[:], in_=o[:, H23:3 * H2],
                                     func=mybir.ActivationFunctionType.Exp)
                nc.vector.tensor_mul(out=o[:, 0:H2], in0=ex_t[:],
                                     in1=epsfull[:, win, :])
                nc.vector.tensor_add(out=o[:, 0:H2], in0=o[:, 0:H2],
                                     in1=o[:, H2:H23])
                nc.sync.dma_start(out=out3[win * P:(win + 1) * P, :], in_=o[:])

            gcount = [0]

            def mk_S(Sw, meta_t, ntot, c):
                nc.vector.tensor_scalar(
                    out=Sw[:, c, :], in0=iota_t[:],
                    scalar1=meta_t[:, c:c + 1],
                    scalar2=meta_t[:, ntot + c:ntot + c + 1],
                    op0=mybir.AluOpType.is_equal, op1=mybir.AluOpType.mult)

            def process_ap_slot(sl, gviews):
                info = m3["apslots"][sl]
                ntot = info["ntot"]
                mc = info["mc"]
                meta_t = metafull[:, 2 * mc:2 * (mc + ntot)]
                acc = psum.tile([P, H23], f32, space="PSUM", tag="acc")
                Sw = spool.tile([P, ntot, P], dt, tag="S")
                # S-builds first: DVE runs a slot ahead of the PE pipeline
                for c in range(ntot):
                    mk_S(Sw, meta_t, ntot, c)
                # hi-src DMA gather early so it overlaps the ap pipeline
                nhi, doff, cnt16 = info["hi"]
                G = None
                if nhi:
                    G = gpool.tile([P, nhi, H23], dt, tag="G")
                    cx = nhi * P if gcount[0] < 2 else min(cnt16, nhi * P)
                    gcount[0] += 1
                    nc.gpsimd.dma_gather(
                        out_ap=G[:], in_ap=supB[:],
                        idxs_ap=idxfull[:, doff:doff + nhi * 8],
                        num_idxs=cx, num_idxs_reg=cx, elem_size=H23,
                        single_packet=False)
                pend = []
                for (h, par, base, nchc) in info["cells"]:
                    for k in range(nchc):
                        pend.append((h, base + P * k, par))
                batches = [pend[i:i + 8] for i in range(0, len(pend), 8)]
                live = []

                def drain(b):
                    gtt, batch, c0 = live[b]
                    gs = gspool.tile([P, len(batch), P], dt, tag="GS")
                    nc.scalar.activation(out=gs[:], in_=gtt[:],
                                         func=mybir.ActivationFunctionType.Copy)
                    for kk in range(len(batch)):
                        c = c0 + kk
                        nc.tensor.matmul(out=acc[:], lhsT=Sw[:, c, :],
                                         rhs=gs[:, kk, :],
                                         start=(c == 0), stop=(c == ntot - 1))

                # transpose batch b+1 is emitted before batch b's copy+matmuls
                # drain so the PE never sits behind the ACT copy latency
                for b, batch in enumerate(batches):
                    gtt = gtp.tile([P, len(batch), P], dt, space="PSUM", tag="GT")
                    for kk, (h, q, par) in enumerate(batch):
                        nc.tensor.transpose(out=gtt[:, kk, :],
                                            in_=gviews[h][:, q:q + P, par],
                                            identity=ident[:])
                    live.append((gtt, batch, 8 * b))
                    if b >= 1:
                        drain(b - 1)
                if batches:
                    drain(len(batches) - 1)
                for k in range(nhi):
                    c = len(pend) + k
                    nc.tensor.matmul(out=acc[:], lhsT=Sw[:, c, :],
                                     rhs=G[:, k, :],
                                     start=(c == 0), stop=(c == ntot - 1))
                finish(sl, acc)

            def process_dma_slot(sl):
                info = m3["apslots"][sl]
                nA, nB, doff, cA16, cB16 = info["dma"]
                ntot = info["ntot"]
                mc = info["mc"]
                meta_t = metafull[:, 2 * mc:2 * (mc + ntot)]
                acc = psum.tile([P, H23], f32, space="PSUM", tag="acc")
                Sw = spool.tile([P, ntot, P], dt, tag="S")
                G = gpool.tile([P, ntot, H23], dt, tag="G")
                for (nch, tab, off2, c16) in ((nA, supA, 0, cA16),
                                              (nB, supB, nA * 8, cB16)):
                    if not nch:
                        continue
                    cx = nch * P if gcount[0] < 2 else min(c16, nch * P)
                    gcount[0] += 1
                    o0 = 0 if off2 == 0 else nA
                    nc.gpsimd.dma_gather(
                        out_ap=G[:, o0:o0 + nch, :], in_ap=tab[:],
                        idxs_ap=idxfull[:, doff + off2:doff + off2 + nch * 8],
                        num_idxs=cx, num_idxs_reg=cx, elem_size=H23,
                        single_packet=False)
                for c in range(ntot):
                    mk_S(Sw, meta_t, ntot, c)
                    nc.tensor.matmul(out=acc[:], lhsT=Sw[:, c, :],
                                     rhs=G[:, c, :],
                                     start=(c == 0), stop=(c == ntot - 1))
                finish(sl, acc)

            for g, grp in enumerate(m3["groups"]):
                gviews = {}
                for h in (0, 1):
                    nidx, ioff = m3["calls"][(g, h)]
                    gapt = appool.tile([P, m3["nidxmax"]], u32, tag="APG")
                    nc.gpsimd.ap_gather(
                        out_ap=gapt[:, 0:nidx], in_ap=t3sb[:, h * APQ:(h + 1) * APQ],
                        idxs_ap=idx3full[:, ioff:ioff + nidx // 16],
                        channels=P, num_elems=APQ, d=1, num_idxs=nidx)
                    gviews[h] = gapt[:].bitcast(dt).rearrange(
                        "p (q two) -> p q two", two=2)
                for sl in grp:
                    process_ap_slot(sl, gviews)
                for sl in m3["dma_after"][g]:
                    process_dma_slot(sl)
    nc.compile()
    return nc


def _get_progs(key, meta, key3, m3):
    ck = (key, key3, L1_DT, SUP1_DT, SUP23_DT)
    if ck not in _PROG_CACHE:
        _PROG_CACHE[ck] = (_build_l1(), _build_l2(meta), _build_l3(m3))
    return _PROG_CACHE[ck]


# ------------------------------------------------------------------- kernel
def _run_spmd(nc, in_maps, tries=4):
    """run_bass_kernel_spmd with retries: the shared device pool occasionally
    needs a few minutes to recover a wedged worker."""
    import time
    for attempt in range(tries):
        try:
            return run_bass_kernel_spmd(nc, in_maps, core_ids=list(range(M)))
        except Exception:
            if attempt == tries - 1:
                raise
            time.sleep(90)


def kernel(x, W1, W2, W3, edge_weight, eps, edge_src, edge_dst):
    x = np.asarray(x, np.float32)
    W1 = np.asarray(W1, np.float32)
    W23 = np.concatenate([np.asarray(W2, np.float32),
                          np.asarray(W3, np.float32)], axis=1)
    eps = np.asarray(eps, np.float32)

    key, meta, eshards = _prep_edges(edge_src, edge_dst, edge_weight)
    key3, m3, eshards3 = _prep_edges3(edge_src, edge_dst, edge_weight)
    nc1, nc2, nc3 = _get_progs(key, meta, key3, m3)

    iota = np.broadcast_to(np.arange(P, dtype=np.float32)[None, :], (P, P))

    # ---- L1: support1 shards
    np1 = NPDT[L1_DT]
    in1 = []
    NPAD = NWIN * P
    for m in range(M):
        xs = np.zeros((NPAD, F_IN), np1)
        xs[:NSH] = x[m * NSH:(m + 1) * NSH].astype(np1)
        xLm = np.ascontiguousarray(
            xs.reshape(NPAD, KCH, P).transpose(2, 0, 1))   # [128, NPAD, KCH]
        in1.append({"xL": xLm, "W1": W1.astype(np1)})
    r1 = _run_spmd(nc1, in1)
    sup1 = np.concatenate([r1.results[m]["s1"][:NSH] for m in range(M)], axis=0)

    # window-slot permutation helpers (slot j on core m = window perm[j])
    def unslot(block, m, H):
        """[NWIN*P, H] slot-blocked -> [NSH, H] node-ordered for core m."""
        perm = eshards[m][2]
        out = np.empty((NSH, H), block.dtype)
        for j in range(NWIN):
            wj = int(perm[j])
            r = min(P, NSH - wj * P)
            out[wj * P:wj * P + r] = block[j * P:j * P + r]
        return out

    def toslot(arr, m):
        """[NSH, H] node-ordered -> [NWIN*P, H] slot-blocked for core m."""
        perm = eshards[m][2]
        out = np.zeros((NWIN * P, arr.shape[1]), arr.dtype)
        for j in range(NWIN):
            wj = int(perm[j])
            r = min(P, NSH - wj * P)
            out[j * P:j * P + r] = arr[wj * P:wj * P + r]
        return out

    # ---- L2: h1 + support23 shards
    np2 = NPDT[SUP1_DT]
    sup1 = sup1.astype(np2)
    in2 = [{"sup1A": sup1[:SPLIT], "sup1B": sup1[N - SPLIT:],
            "eidx": eshards[m][0], "emeta": eshards[m][1],
            "W23": W23, "iota": iota.astype(np2)}
           for m in range(M)]
    r2 = _run_spmd(nc2, in2)
    NPAIR = (NWIN + 1) // 2
    sup23_parts = []
    for m in range(M):
        pr = r2.results[m]["s23"].reshape(NPAIR, P, 2, H23)
        sl = np.empty((NWIN * P, H23), pr.dtype)
        for j in range(NWIN):
            sl[j * P:(j + 1) * P] = pr[j // 2, :, j % 2, :]
        sup23_parts.append(unslot(sl, m, H23))
    sup23 = np.concatenate(sup23_parts, axis=0)

    # ---- L3: mu, logvar, z shards
    np3 = NPDT[SUP23_DT]
    sup23 = sup23.astype(np3)
    t = sup23[:APN].reshape(APE, 2, H23)
    lo = t[:, 0, :].view(np.uint16).astype(np.uint32)
    hi = t[:, 1, :].view(np.uint16).astype(np.uint32)
    T3 = np.ascontiguousarray(((hi << 16) | lo).T)   # [128, APE] u32
    in3 = [{"sup23A": sup23[:SPLIT], "sup23B": sup23[N - SPLIT:],
            "eidx": eshards3[m][1], "eidx3": eshards3[m][0],
            "emeta": eshards3[m][2], "t3": T3,
            "iota": iota.astype(np3),
            "epss": np.ascontiguousarray(
                toslot(eps[m * NSH:(m + 1) * NSH], m)
                .reshape(NWIN, P, H2).transpose(1, 0, 2)
                .reshape(P, NWIN * H2)).astype(np.float16)}
           for m in range(M)]
    r3 = _run_spmd(nc3, in3)
    outs = [unslot(r3.results[m]["out3"], m, 3 * H2) for m in range(M)]
    full = np.concatenate(outs, axis=0)
    z, mu, logvar = full[:, 0:H2], full[:, H2:H23], full[:, H23:3 * H2]
    return (np.ascontiguousarray(z), np.ascontiguousarray(mu),
            np.ascontiguousarray(logvar))



# revision 29
# speedup vs baseline: 1.0191x; 1.0106x over previous
"""GCN-VAE encoder (2-layer GCN + reparameterize) on 8 Trainium2 NeuronCores.

Strategy (per the dst-sharding hint):
  - Nodes are sharded across the 8 cores (6250 rows each); edges are
    partitioned by dst and sorted by dst within each core.
  - Layer matmuls (x@W1, h1@[W2|W3]) run on the node shard each core owns.
  - The sparse aggregation out[d] = sum_{(s,d) in E} w * feat[s] is computed
    per 128-dst-node "window": edges are chunked into groups of 128, features
    feat[src] are fetched with dma_gather row gathers (two per window — the
    int16 gather index forces a <32768 split of the feature table), and the
    segment-sum is a PE matmul acc += S^T @ G with a one-hot matrix
    S[e, dst_local[e]] = w_e built on the vector engine.
  - Cross-core exchange of the full feature tables (support1, support23)
    happens through host round-trips between three SPMD launches (no
    on-device collectives needed):
      L1: support1_shard = x_shard @ W1
      L2: h1 = relu(spmm(support1_full));  support23_shard = h1 @ [W2|W3]
      L3: [mu|logvar] = relu(spmm(support23_full)); z = eps*exp(logvar)+mu
"""

import sys

for _p in ("/opt/trn_rl_repo", "/root/.axon_site/_ro/trn_rl_repo"):
    if _p not in sys.path:
        sys.path.append(_p)

import numpy as np
import ml_dtypes

import concourse.mybir as mybir
import concourse.tile as tile
from concourse import bacc
from concourse.bass_utils import run_bass_kernel_spmd
from concourse.masks import make_identity

# ---- problem constants (hardcoded per harness contract) ----
N, E, F_IN, H1, H2 = 50000, 1600000, 512, 256, 64
H23 = 2 * H2                      # concat(mu, logvar) feature width
M = 8                             # cores
NSH = N // M                      # nodes per core
P = 128                           # partitions / window size / edge chunk
NWIN = (NSH + P - 1) // P         # dst windows per core (49)
KCH = F_IN // P                   # k-chunks for layer-1 matmul (4)
SPLIT = 32768                     # int16 gather-index limit

f32 = mybir.dt.float32
f16 = mybir.dt.float16
i16 = mybir.dt.int16
u32 = mybir.dt.uint32

# L3 hybrid-lane parameters: edges of the first AP_SLOTS dst-slots whose src
# is below APN are gathered on-chip (gpsimd ap_gather from an SBUF-resident
# u32 node-pair-packed transposed sup23 table) instead of via DMA descriptors;
# this splits the gather load across the Pool engine and the DMA engines.
APN = 28672                       # nodes covered by the ap-lane pair table
APE = APN // 2                    # pair elements (u32 cols)
APQ = APE // 2                    # gather-view half (table-term balance)
AP_SLOTS = 24                     # slots using the ap-lane (rest pure DMA)
GRP = 6                           # ap slots per gather group

DT = {"f32": mybir.dt.float32, "f32r": mybir.dt.float32r,
      "f16": mybir.dt.float16, "bf16": mybir.dt.bfloat16}
NPDT = {"f32": np.float32, "f32r": np.float32,
        "f16": np.float16, "bf16": ml_dtypes.bfloat16}

# dtype ladder (accuracy/speed): "f32" exact, "f32r" single-pass PE fp32,
# "f16"/"bf16" half-width gather tables.
L1_DT = "f16"                     # x/W1 matmul operand dtype
SUP1_DT = "f16"                  # layer-1 feature table + S dtype
SUP23_DT = "f16"                  # layer-2/3 feature table + S dtype

_PROG_CACHE: dict = {}


# ---------------------------------------------------------------- host prep
def _wrap16(arr):
    """int16 gather-index layout: ordinal i -> [i%16, i//16], replicated to
    128 partitions (8 Q7 cores x 16)."""
    w = arr.astype(np.int16).reshape(-1, 16).T
    return np.tile(w, (8, 1))


def _prep_edges(edge_src, edge_dst, edge_weight):
    """Partition edges by dst shard, sort by (window, src-half), lay out
    per-window chunk tiles.

    Device arrays per core:
      EIDX [128, (totA+totB)*8] int16 — per window [A idxs | B idxs] wrapped
      EMETA [128, 2*totc] f32 — per window [dst_local (nw) | weight (nw)]
    Returns (key, meta, shards); meta holds the static chunk structure
    (identical across cores by construction)."""
    edge_src = np.asarray(edge_src).astype(np.int64)
    edge_dst = np.asarray(edge_dst).astype(np.int64)
    edge_weight = np.asarray(edge_weight).astype(np.float32)

    percore = []
    cntA = np.zeros((M, NWIN), np.int64)
    cntB = np.zeros((M, NWIN), np.int64)
    cntT = np.zeros((M, NWIN), np.int64)
    perms = []
    for m in range(M):
        sel = (edge_dst >= m * NSH) & (edge_dst < (m + 1) * NSH)
        d = edge_dst[sel] - m * NSH
        s = edge_src[sel]
        w = edge_weight[sel]
        win0 = d >> 7
        # slot-balance: rank windows by edge count so slot j holds each
        # core's j-th-largest window -> cross-core max padding shrinks
        wcnt = np.bincount(win0, minlength=NWIN)
        perm = np.argsort(-wcnt, kind="stable")           # slot -> window
        inv = np.empty(NWIN, np.int64)
        inv[perm] = np.arange(NWIN)                       # window -> slot
        perms.append(perm)
        win = inv[win0]                                   # slot index per edge
        klass = np.where(s < N - SPLIT, 0,
                         np.where(s < SPLIT, 1, 2))       # A-only/flex/B-only
        order = np.lexsort((klass, win))
        d, s, w, win, klass = d[order], s[order], w[order], win[order], klass[order]
        dloc = d - (perm[win] << 7)                       # dst_local in window
        for h, cnt in ((0, cntA), (2, cntB)):
            msk = klass == h
            cnt[m] = np.bincount(win[msk], minlength=NWIN)
        cntT[m] = np.bincount(win, minlength=NWIN)
        percore.append((dloc, s, w, win, klass))

    # chunk budget per slot: total rounded up, A sized to forced-A max,
    # flex edges fill A chunks to capacity before B
    tots = np.stack([np.bincount(pc[3], minlength=NWIN) for pc in percore])
    ncwT = np.maximum(2, -(-tots.max(axis=0) // P))
    ncwA = np.maximum(1, -(-cntA.max(axis=0) // P))
    while True:
        bad = (cntB.max(axis=0) > (ncwT - ncwA) * P)
        if not bad.any():
            break
        ncwT[bad] += 1
    ncwB = ncwT - ncwA
    ncw = ncwT
    offs = np.concatenate([[0], np.cumsum(ncw)])
    offsA = np.concatenate([[0], np.cumsum(ncwA)])
    offsB = np.concatenate([[0], np.cumsum(ncwB)])
    totc = int(offs[-1])
    totA, totB = int(offsA[-1]), int(offsB[-1])
    # exact (16-rounded) per-slot gather counts: cross-core max of edges
    # actually landing in each A/B half -> num_idxs below chunk-granular pad
    perA = np.minimum(cntT, ncwA[None, :] * P)
    exactA = (-(-perA.max(axis=0) // 16) * 16).astype(np.int64)
    exactB = (-(-(cntT - perA).max(axis=0) // 16) * 16).astype(np.int64)

    shards = []
    for m in range(M):
        dloc, s, w, win, klass = percore[m]
        DSTL = np.zeros((P, totc), np.float32)
        WGT = np.zeros((P, totc), np.float32)
        srcA = np.zeros(totA * P, np.int64)
        srcB = np.zeros(totB * P, np.int64)
        # edges are slot-major, class-ordered (A-only, flex, B-only): the
        # first capA go to half A (flex spills into A until its chunks are
        # full), the rest to half B
        wcnt = np.bincount(win, minlength=NWIN)
        starts = np.concatenate([[0], np.cumsum(wcnt)])[:-1]
        jall = np.arange(len(dloc)) - starts[win]          # rank within slot
        capA = ncwA[win] * P
        nAB = np.minimum(wcnt, ncwA * P)                   # A edges per slot
        toA = jall < capA
        for h, offsH, srcH, base_extra, sub in (
                (True, offsA, srcA, 0, 0),
                (False, offsB, srcB, None, SPLIT)):
            msk = toA if h else ~toA
            dh, sh, wh, winh = dloc[msk], s[msk], w[msk], win[msk]
            j = jall[msk] if h else jall[msk] - nAB[winh]
            rows = j % P
            base = offs[winh] + (0 if h else ncwA[winh])
            cols = base + j // P
            DSTL[rows, cols] = dh
            WGT[rows, cols] = wh
            srcH[offsH[winh] * P + j] = sh - (0 if h else N - SPLIT)
        IDXA = _wrap16(srcA)                               # [128, totA*8]
        IDXB = _wrap16(srcB)
        EIDX = np.zeros((P, (totA + totB) * 8), np.int16)
        EMETA = np.zeros((P, 2 * totc), np.float32)
        for t in range(NWIN):
            nA, nB = int(ncwA[t]), int(ncwB[t])
            co = (int(offsA[t]) + int(offsB[t])) * 8
            EIDX[:, co:co + nA * 8] = IDXA[:, offsA[t] * 8:(offsA[t] + nA) * 8]
            EIDX[:, co + nA * 8:co + (nA + nB) * 8] = \
                IDXB[:, offsB[t] * 8:(offsB[t] + nB) * 8]
            o = int(offs[t])
            EMETA[:, 2 * o:2 * o + (nA + nB)] = DSTL[:, o:o + nA + nB]
            EMETA[:, 2 * o + nA + nB:2 * (o + nA + nB)] = WGT[:, o:o + nA + nB]
        shards.append((EIDX, EMETA, perms[m]))

    key = (tuple(int(v) for v in ncwA) + tuple(int(v) for v in ncwB)
           + tuple(int(v) for v in exactA) + tuple(int(v) for v in exactB))
    meta = (tuple(int(v) for v in ncwA), tuple(int(v) for v in ncwB),
            offs, offsA, offsB, totc, totA, totB,
            tuple(int(v) for v in exactA), tuple(int(v) for v in exactB))
    return key, meta, shards


def _prep_edges3(edge_src, edge_dst, edge_weight):
    """L3 hybrid-lane edge layout.

    Slots 0..AP_SLOTS-1: edges with src < APN go to the ap-lane, laid out per
    (gather-group, table-half) call as 16-rounded (slot, parity) cell blocks
    of pair indices; src >= APN edges stay on the DMA lane (supB gather).
    Slots AP_SLOTS..: plain DMA A/B gathers (no flex balancing).
    Chunk metadata (dst_local | weight) is emitted slot-major into EMETA3."""
    edge_src = np.asarray(edge_src).astype(np.int64)
    edge_dst = np.asarray(edge_dst).astype(np.int64)
    edge_weight = np.asarray(edge_weight).astype(np.float32)

    # per-core per-slot edge lists, same slot permutation as _prep_edges
    cores = []
    for m in range(M):
        sel = (edge_dst >= m * NSH) & (edge_dst < (m + 1) * NSH)
        d = edge_dst[sel] - m * NSH
        s = edge_src[sel]
        w = edge_weight[sel]
        win0 = d >> 7
        wcnt = np.bincount(win0, minlength=NWIN)
        perm = np.argsort(-wcnt, kind="stable")
        inv = np.empty(NWIN, np.int64)
        inv[perm] = np.arange(NWIN)
        win = inv[win0]
        dloc = d - (perm[win] << 7)
        # cell id: ap slots: 0..3 = (half,parity), 4 = hi(DMA); dma slots: 5=A, 6=B
        is_ap = (win < AP_SLOTS) & (s < APN)
        cell = np.where(is_ap, ((s >> 1) >= APQ) * 2 + (s & 1),
                        np.where(win < AP_SLOTS, 4, np.where(s < SPLIT, 5, 6)))
        order = np.lexsort((cell, win))
        cores.append((dloc[order], s[order], w[order], win[order], cell[order]))

    # cross-core max counts per (slot, cell)
    cnt = np.zeros((M, NWIN, 7), np.int64)
    for m, (dloc, s, w, win, cell) in enumerate(cores):
        np.add.at(cnt[m], (win, cell), 1)
    cmax = cnt.max(axis=0)                       # [NWIN, 7]

    r16 = lambda v: -(-int(v) // 16) * 16
    nch_of = lambda v: max(0, -(-int(v) // P))

    groups = [list(range(g * GRP, min((g + 1) * GRP, AP_SLOTS)))
              for g in range(-(-AP_SLOTS // GRP))]
    dma_slots = list(range(AP_SLOTS, NWIN))
    nsp = len(groups)
    dma_after = [dma_slots[(len(dma_slots) * g) // nsp:
                           (len(dma_slots) * (g + 1)) // nsp] for g in range(nsp)]

    # ---- static layout
    apslots = {}
    calls = {}
    ic3 = 0
    for g, grp in enumerate(groups):
        for h in (0, 1):
            pos = 0
            for sl in grp:
                for par in (0, 1):
                    c = 2 * h + par
                    raw = cmax[sl, c]
                    nchc = nch_of(raw)
                    apslots.setdefault(sl, {"cells": [], "hi": None})
                    # blocks are 128-chunk aligned: the PE transpose's strided
                    # lhsT view silently corrupts at non-128 base offsets
                    apslots[sl]["cells"].append((h, par, pos, nchc))
                    pos += P * nchc
            nidx = max(pos, 16)
            calls[(g, h)] = (nidx, ic3)
            ic3 += nidx // 16

    icd = 0
    totc = 0
    for sl in range(NWIN):
        if sl < AP_SLOTS:
            nhi = nch_of(cmax[sl, 4])
            apslots[sl]["hi"] = (nhi, icd, r16(cmax[sl, 4]))
            ncells = sum(nc_ for (_, _, _, nc_) in apslots[sl]["cells"])
            apslots[sl]["ntot"] = ncells + nhi
            apslots[sl]["mc"] = totc
            totc += apslots[sl]["ntot"]
            icd += nhi * 8
        else:
            nA, nB = nch_of(cmax[sl, 5]), nch_of(cmax[sl, 6])
            apslots[sl] = {"dma": (nA, nB, icd, r16(cmax[sl, 5]), r16(cmax[sl, 6])),
                           "mc": totc, "ntot": nA + nB}
            totc += nA + nB
            icd += (nA + nB) * 8
    nidxmax = max(v[0] for v in calls.values())

    m3 = {"groups": groups, "dma_after": dma_after, "calls": calls,
          "apslots": apslots, "ic3": ic3, "icd": icd, "totc": totc,
          "nidxmax": nidxmax}

    # ---- per-core arrays
    shards3 = []
    for m, (dloc, s, w, win, cell) in enumerate(cores):
        EIDX3 = np.zeros((P, ic3), np.int16)
        EIDXD = np.zeros((P, icd), np.int16)
        EMETA3 = np.zeros((P, 2 * totc), np.float32)
        # per (slot, cell) edge slices (cells contiguous per lexsort)
        starts = {}
        k0 = 0
        keyarr = win * 8 + cell
        bounds = np.flatnonzero(np.diff(keyarr)) + 1
        segs = np.split(np.arange(len(win)), bounds)
        for seg in segs:
            if len(seg) == 0:
                continue
            starts[(int(win[seg[0]]), int(cell[seg[0]]))] = seg

        def fill_meta(sl, ci0, nchc, seg):
            ntot = apslots[sl]["ntot"]
            mc = apslots[sl]["mc"]
            if seg is None or nchc == 0:
                return
            j = np.arange(len(seg))
            rows, cols = j % P, ci0 + j // P
            EMETA3[rows, 2 * mc + cols] = dloc[seg]
            EMETA3[rows, 2 * mc + ntot + cols] = w[seg]

        for g, grp in enumerate(groups):
            for h in (0, 1):
                nidx, ioff = calls[(g, h)]
                buf = np.zeros(nidx, np.int64)
                for sl in grp:
                    ci = 0
                    for (hh, par, pos, nchc) in apslots[sl]["cells"]:
                        seg = starts.get((sl, 2 * hh + par))
                        if hh == h and seg is not None:
                            buf[pos:pos + len(seg)] = (s[seg] >> 1) - h * APQ
                        if hh == h:
                            pass
                        ci += nchc
                EIDX3[:, ioff:ioff + nidx // 16] = _wrap16(buf)
        # meta for ap cells + hi, and dma blocks
        for sl in range(NWIN):
            info = apslots[sl]
            if sl < AP_SLOTS:
                ci = 0
                for (hh, par, pos, nchc) in info["cells"]:
                    fill_meta(sl, ci, nchc, starts.get((sl, 2 * hh + par)))
                    ci += nchc
                nhi, doff, cnt16 = info["hi"]
                seg = starts.get((sl, 4))
                fill_meta(sl, ci, nhi, seg)
                if nhi:
                    buf = np.zeros(nhi * P, np.int64)
                    if seg is not None:
                        buf[:len(seg)] = s[seg] - (N - SPLIT)
                    EIDXD[:, doff:doff + nhi * 8] = _wrap16(buf)
            else:
                nA, nB, doff, cA16, cB16 = info["dma"]
                ci = 0
                for cc, nchc, sub, off2 in ((5, nA, 0, 0), (6, nB, N - SPLIT, nA * 8)):
                    seg = starts.get((sl, cc))
                    fill_meta(sl, ci, nchc, seg)
                    if nchc:
                        buf = np.zeros(nchc * P, np.int64)
                        if seg is not None:
                            buf[:len(seg)] = s[seg] - sub
                        EIDXD[:, doff + off2:doff + off2 + nchc * 8] = _wrap16(buf)
                    ci += nchc
        shards3.append((EIDX3, EIDXD, EMETA3))

    key3 = (ic3, icd, totc, nidxmax,
            tuple(apslots[sl]["ntot"] for sl in range(NWIN)),
            tuple(v[0] for v in calls.values()))
    return key3, m3, shards3


# ------------------------------------------------------------- bass builders
def _mk_nc():
    return bacc.Bacc("TRN2", target_bir_lowering=False, debug=False)


def _build_l1():
    """support1_shard[6250,256] = x_shard @ W1.

    xL is host-prepared as [128, NSH, KCH] (xL[p,n,k] = x[n, k*128+p]) so the
    whole shard loads SBUF-resident with four big contiguous DMAs; matmuls
    read stationary tiles straight out of it."""
    dt = DT[L1_DT]
    nc = _mk_nc()
    odt = DT[SUP1_DT]
    xL = nc.dram_tensor("xL", [P, NWIN * P, KCH], dt, kind="ExternalInput")
    W1 = nc.dram_tensor("W1", [F_IN, H1], dt, kind="ExternalInput")
    s1 = nc.dram_tensor("s1", [NWIN * P, H1], odt, kind="ExternalOutput")
    s1r = s1[:].rearrange("(t p) h -> p t h", p=P)          # [128, NWIN, H1]

    NSPLIT = 8
    NPAD = NWIN * P
    spans = [(NPAD * i // NSPLIT, NPAD * (i + 1) // NSPLIT) for i in range(NSPLIT)]
    with tile.TileContext(nc) as tc:
        with tc.tile_pool(name="const", bufs=1) as cpool, \
             tc.tile_pool(name="sbuf", bufs=6) as pool, \
             tc.tile_pool(name="psum", bufs=8, space="PSUM") as psum:
            w1c = cpool.tile([P, KCH, H1], dt)
            nc.sync.dma_start(out=w1c[:],
                              in_=W1[:].rearrange("(k p) n -> p k n", p=P))
            xfull = cpool.tile([P, NWIN * P, KCH], dt)
            for a, b in spans:
                nc.sync.dma_start(out=xfull[:, a:b, :], in_=xL[:, a:b, :])
            ofull = cpool.tile([P, NWIN, H1], odt)
            OSEC = 4
            sec = [(NWIN * i // OSEC, NWIN * (i + 1) // OSEC) for i in range(OSEC)]
            si = 0
            for t in range(NWIN):
                acc = psum.tile([P, H1], f32, space="PSUM", tag="acc")
                for k in range(KCH):
                    nc.tensor.matmul(
                        out=acc[:],
                        lhsT=xfull[:, t * P:(t + 1) * P, k],
                        rhs=w1c[:, k, :],
                        start=(k == 0), stop=(k == KCH - 1))
                nc.scalar.activation(out=ofull[:, t, :], in_=acc[:],
                                     func=mybir.ActivationFunctionType.Copy)
                if t + 1 == sec[si][1]:
                    a, b = sec[si]
                    nc.sync.dma_start(out=s1r[:, a:b, :], in_=ofull[:, a:b, :])
                    si += 1
    nc.compile()
    return nc


def _spmm_windows(nc, pool, psum, gpool, cpool, supA, supB, eidx, emeta,
                  iota_t, meta, H, dt, per_window_out):
    """Shared spmm structure: for each window slot produce PSUM [128, H]
    segment sum, then call per_window_out(win, rows, acc_psum). Edge index
    and metadata arrays are loaded SBUF-resident once up front."""
    ncwA, ncwB, offs, offsA, offsB, totc, totA, totB, exactA, exactB = meta
    metafull = cpool.tile([P, 2 * totc], f32)
    idxfull = cpool.tile([P, (totA + totB) * 8], i16)
    bnds = [0, NWIN // 4, NWIN // 2, (3 * NWIN) // 4, NWIN]
    for a, b in zip(bnds[:-1], bnds[1:]):
        mo, mo2 = 2 * int(offs[a]), 2 * int(offs[b])
        nc.sync.dma_start(out=metafull[:, mo:mo2], in_=emeta[:, mo:mo2])
        io = (int(offsA[a]) + int(offsB[a])) * 8
        io2 = (int(offsA[b]) + int(offsB[b])) * 8
        nc.sync.dma_start(out=idxfull[:, io:io2], in_=eidx[:, io:io2])
    for win in range(NWIN):
        rows = P
        nA, nB = ncwA[win], ncwB[win]
        nw = nA + nB
        off = int(offs[win])
        co = (int(offsA[win]) + int(offsB[win])) * 8
        meta_t = metafull[:, 2 * off:2 * (off + nw)]

        G = gpool.tile([P, nw, H], dt, tag="G")
        # exact counts shave chunk-granular padding; first pool rotation uses
        # the full padded count so no G rows are ever read uninitialized
        cA = nA * P if win < 4 else min(int(exactA[win]), nA * P)
        cB = nB * P if win < 4 else min(int(exactB[win]), nB * P)
        nc.gpsimd.dma_gather(
            out_ap=G[:, 0:nA, :], in_ap=supA[:],
            idxs_ap=idxfull[:, co:co + nA * 8],
            num_idxs=cA, num_idxs_reg=cA, elem_size=H,
            single_packet=False)
        nc.gpsimd.dma_gather(
            out_ap=G[:, nA:nw, :], in_ap=supB[:],
            idxs_ap=idxfull[:, co + nA * 8:co + nw * 8],
            num_idxs=cB, num_idxs_reg=cB, elem_size=H,
            single_packet=False)

        acc = psum.tile([P, H], f32, space="PSUM", tag="acc")
        S = pool.tile([P, nw, P], dt, tag="S")
        for c in range(nw):
            nc.vector.tensor_scalar(
                out=S[:, c, :], in0=iota_t[:],
                scalar1=meta_t[:, c:c + 1], scalar2=meta_t[:, nw + c:nw + c + 1],
                op0=mybir.AluOpType.is_equal, op1=mybir.AluOpType.mult)
            nc.tensor.matmul(
                out=acc[:],
                lhsT=S[:, c, :],
                rhs=G[:, c, :],
                start=(c == 0), stop=(c == nw - 1))
        per_window_out(win, rows, acc)


def _decl_spmm_inputs(nc, meta, H, dt, supname):
    totc, totA, totB = meta[5], meta[6], meta[7]
    supA = nc.dram_tensor(supname + "A", [SPLIT, H], dt, kind="ExternalInput")
    supB = nc.dram_tensor(supname + "B", [SPLIT, H], dt, kind="ExternalInput")
    eidx = nc.dram_tensor("eidx", [P, (totA + totB) * 8], i16, kind="ExternalInput")
    emeta = nc.dram_tensor("emeta", [P, 2 * totc], f32, kind="ExternalInput")
    iota_h = nc.dram_tensor("iota", [P, P], dt, kind="ExternalInput")
    return supA, supB, eidx, emeta, iota_h


def _build_l2(meta):
    """h1 = relu(spmm(support1)); support23_shard = h1 @ W23."""
    dt = DT[SUP1_DT]
    nc = _mk_nc()
    supA, supB, eidx, emeta, iota_h = _decl_spmm_inputs(nc, meta, H1, dt, "sup1")
    W23 = nc.dram_tensor("W23", [H1, H23], f32, kind="ExternalInput")
    odt = DT[SUP23_DT]
    NPAIR = (NWIN + 1) // 2
    s23 = nc.dram_tensor("s23", [NPAIR * P, 2 * H23], odt, kind="ExternalOutput")

    with tile.TileContext(nc) as tc:
        with tc.tile_pool(name="const", bufs=1) as cpool, \
             tc.tile_pool(name="sbuf", bufs=3) as pool, \
             tc.tile_pool(name="small", bufs=8) as spool, \
             tc.tile_pool(name="gpool", bufs=4) as gpool, \
             tc.tile_pool(name="psum", bufs=3, space="PSUM") as psum, \
             tc.tile_pool(name="psum2", bufs=2, space="PSUM") as psum2:
            iota_t = cpool.tile([P, P], dt)
            nc.sync.dma_start(out=iota_t[:], in_=iota_h[:])
            ident = cpool.tile([P, P], f32)
            make_identity(nc, ident[:])
            w23c = cpool.tile([P, H1 // P, H23], f32)
            nc.sync.dma_start(out=w23c[:],
                              in_=W23[:].rearrange("(k p) n -> p k n", p=P))

            opair_box = [None]

            def finish(win, rows, acc):
                h1 = pool.tile([P, H1], f32, tag="h1")
                nc.scalar.activation(out=h1[:], in_=acc[:],
                                     func=mybir.ActivationFunctionType.Relu)
                ps23 = psum2.tile([P, H23], f32, space="PSUM", tag="ps23")
                for fh in range(H1 // P):
                    tp = psum2.tile([P, P], f32, space="PSUM", tag="tp")
                    nc.tensor.transpose(out=tp[:], in_=h1[:, fh * P:(fh + 1) * P],
                                        identity=ident[:])
                    tps = pool.tile([P, P], f32, tag="tps")
                    nc.vector.tensor_copy(out=tps[:], in_=tp[:])
                    nc.tensor.matmul(
                        out=ps23[:],
                        lhsT=tps[:],
                        rhs=w23c[:, fh, :],
                        start=(fh == 0), stop=(fh == H1 // P - 1))
                if win % 2 == 0:
                    op_t = pool.tile([P, 2, H23], odt, tag="opair")
                    opair_box[0] = op_t
                opair = opair_box[0]
                nc.scalar.activation(out=opair[:, win % 2, :], in_=ps23[:],
                                     func=mybir.ActivationFunctionType.Copy)
                pb = win // 2
                if win % 2 == 1:
                    nc.sync.dma_start(out=s23[pb * P:(pb + 1) * P, :],
                                      in_=opair[:])
                elif win == NWIN - 1:
                    nc.sync.dma_start(out=s23[pb * P:(pb + 1) * P, 0:H23],
                                      in_=opair[:, 0, :])

            _spmm_windows(nc, spool, psum, gpool, cpool, supA, supB, eidx,
                          emeta, iota_t, meta, H1, dt, finish)
    nc.compile()
    return nc


def _build_l3(m3):
    """[mu|logvar] = relu(spmm(support23)); z = eps*exp(logvar)+mu.

    Hybrid gather: ap-lane slots pull low-src features from an SBUF-resident
    u32 pair-packed transposed table via gpsimd ap_gather (features land on
    partitions, so each 128-edge chunk is PE-transposed and staged through an
    ACT batch copy before the S matmul); remaining edges use DMA dma_gather."""
    dt = DT[SUP23_DT]
    nc = _mk_nc()
    supA = nc.dram_tensor("sup23A", [SPLIT, H23], dt, kind="ExternalInput")
    supB = nc.dram_tensor("sup23B", [SPLIT, H23], dt, kind="ExternalInput")
    eidx = nc.dram_tensor("eidx", [P, m3["icd"]], i16, kind="ExternalInput")
    eidx3 = nc.dram_tensor("eidx3", [P, m3["ic3"]], i16, kind="ExternalInput")
    emeta = nc.dram_tensor("emeta", [P, 2 * m3["totc"]], f32, kind="ExternalInput")
    t3 = nc.dram_tensor("t3", [P, APE], u32, kind="ExternalInput")
    iota_h = nc.dram_tensor("iota", [P, P], dt, kind="ExternalInput")
    epss = nc.dram_tensor("epss", [P, NWIN * H2], f16, kind="ExternalInput")
    out3 = nc.dram_tensor("out3", [NWIN * P, 3 * H2], f32, kind="ExternalOutput")
    totc = m3["totc"]

    with tile.TileContext(nc) as tc:
        with tc.tile_pool(name="const", bufs=1) as cpool, \
             tc.tile_pool(name="sbuf", bufs=3) as pool, \
             tc.tile_pool(name="spool", bufs=2) as spool, \
             tc.tile_pool(name="gpool", bufs=2) as gpool, \
             tc.tile_pool(name="appool", bufs=2) as appool, \
             tc.tile_pool(name="gspool", bufs=2) as gspool, \
             tc.tile_pool(name="psum", bufs=6, space="PSUM") as psum, \
             tc.tile_pool(name="gtpsum", bufs=2, space="PSUM") as gtp:
            iota_t = cpool.tile([P, P], dt)
            nc.sync.dma_start(out=iota_t[:], in_=iota_h[:])
            ident = cpool.tile([P, P], dt)
            make_identity(nc, ident[:])
            epsfull = cpool.tile([P, NWIN, H2], f16)
            nc.sync.dma_start(out=epsfull[:], in_=epss[:])
            t3sb = cpool.tile([P, APE], u32)
            nc.sync.dma_start(out=t3sb[:, :APE // 2], in_=t3[:, :APE // 2])
            nc.sync.dma_start(out=t3sb[:, APE // 2:], in_=t3[:, APE // 2:])
            metafull = cpool.tile([P, 2 * totc], f32)
            idxfull = cpool.tile([P, m3["icd"]], i16)
            idx3full = cpool.tile([P, m3["ic3"]], i16)
            for t, src, n in ((metafull, emeta, 2 * totc),
                              (idxfull, eidx, m3["icd"]),
                              (idx3full, eidx3, m3["ic3"])):
                for a, b in ((0, n // 2), (n // 2, n)):
                    if b > a:
                        nc.sync.dma_start(out=t[:, a:b], in_=src[:, a:b])

            def finish(win, acc):
                o = pool.tile([P, 3 * H2], f32, tag="o3")
                nc.scalar.activation(out=o[:, H2:H23], in_=acc[:, 0:H2],
                                     func=mybir.ActivationFunctionType.Relu)
                nc.scalar.activation(out=o[:, H23:3 * H2], in_=acc[:, H2:H23],
                                     func=mybir.ActivationFunctionType.Relu)
                ex_t = pool.tile([P, H2], f32, tag="ex")
                nc.scalar.activation(out=ex_t[:], in_=o[:, H23:3 * H2],
                                     func=mybir.ActivationFunctionType.Exp)
                nc.vector.tensor_mul(out=o[:, 0:H2], in0=ex_t[:],
                                     in1=epsfull[:, win, :])
                nc.vector.tensor_add(out=o[:, 0:H2], in0=o[:, 0:H2],
                                     in1=o[:, H2:H23])
                nc.sync.dma_start(out=out3[win * P:(win + 1) * P, :], in_=o[:])

            gcount = [0]

            def mk_S(Sw, meta_t, ntot, c):
                nc.vector.tensor_scalar(
                    out=Sw[:, c, :], in0=iota_t[:],
                    scalar1=meta_t[:, c:c + 1],
                    scalar2=meta_t[:, ntot + c:ntot + c + 1],
                    op0=mybir.AluOpType.is_equal, op1=mybir.AluOpType.mult)

            def process_ap_slot(sl, gviews):
                info = m3["apslots"][sl]
                ntot = info["ntot"]
                mc = info["mc"]
                meta_t = metafull[:, 2 * mc:2 * (mc + ntot)]
                acc = psum.tile([P, H23], f32, space="PSUM", tag="acc")
                Sw = spool.tile([P, ntot, P], dt, tag="S")
                # S-builds first: DVE runs a slot ahead of the PE pipeline
                for c in range(ntot):
                    mk_S(Sw, meta_t, ntot, c)
                # hi-src DMA gather early so it overlaps the ap pipeline
                nhi, doff, cnt16 = info["hi"]
                G = None
                if nhi:
                    G = gpool.tile([P, nhi, H23], dt, tag="G")
                    cx = nhi * P if gcount[0] < 2 else min(cnt16, nhi * P)
                    gcount[0] += 1
                    nc.gpsimd.dma_gather(
                        out_ap=G[:], in_ap=supB[:],
                        idxs_ap=idxfull[:, doff:doff + nhi * 8],
                        num_idxs=cx, num_idxs_reg=cx, elem_size=H23,
                        single_packet=False)
                pend = []
                for (h, par, base, nchc) in info["cells"]:
                    for k in range(nchc):
                        pend.append((h, base + P * k, par))
                batches = [pend[i:i + 8] for i in range(0, len(pend), 8)]
                live = []

                def drain(b):
                    gtt, batch, c0 = live[b]
                    gs = gspool.tile([P, len(batch), P], dt, tag="GS")
                    nc.scalar.activation(out=gs[:], in_=gtt[:],
                                         func=mybir.ActivationFunctionType.Copy)
                    for kk in range(len(batch)):
                        c = c0 + kk
                        nc.tensor.matmul(out=acc[:], lhsT=Sw[:, c, :],
                                         rhs=gs[:, kk, :],
                                         start=(c == 0), stop=(c == ntot - 1))

                # transpose batch b+1 is emitted before batch b's copy+matmuls
                # drain so the PE never sits behind the ACT copy latency
                for b, batch in enumerate(batches):
                    gtt = gtp.tile([P, len(batch), P], dt, space="PSUM", tag="GT")
                    for kk, (h, q, par) in enumerate(batch):
                        nc.tensor.transpose(out=gtt[:, kk, :],
                                            in_=gviews[h][:, q:q + P, par],
                                            identity=ident[:])
                    live.append((gtt, batch, 8 * b))
                    if b >= 1:
                        drain(b - 1)
                if batches:
                    drain(len(batches) - 1)
                for k in range(nhi):
                    c = len(pend) + k
                    nc.tensor.matmul(out=acc[:], lhsT=Sw[:, c, :],
                                     rhs=G[:, k, :],
                                     start=(c == 0), stop=(c == ntot - 1))
                finish(sl, acc)

            def process_dma_slot(sl):
                info = m3["apslots"][sl]
                nA, nB, doff, cA16, cB16 = info["dma"]
                ntot = info["ntot"]
                mc = info["mc"]
                meta_t = metafull[:, 2 * mc:2 * (mc + ntot)]
                acc = psum.tile([P, H23], f32, space="PSUM", tag="acc")
                Sw = spool.tile([P, ntot, P], dt, tag="S")
                G = gpool.tile([P, ntot, H23], dt, tag="G")
                for (nch, tab, off2, c16) in ((nA, supA, 0, cA16),
                                              (nB, supB, nA * 8, cB16)):
                    if not nch:
                        continue
                    cx = nch * P if gcount[0] < 2 else min(c16, nch * P)
                    gcount[0] += 1
                    o0 = 0 if off2 == 0 else nA
                    nc.gpsimd.dma_gather(
                        out_ap=G[:, o0:o0 + nch, :], in_ap=tab[:],
                        idxs_ap=idxfull[:, doff + off2:doff + off2 + nch * 8],
                        num_idxs=cx, num_idxs_reg=cx, elem_size=H23,
                        single_packet=False)
                for c in range(ntot):
                    mk_S(Sw, meta_t, ntot, c)
                    nc.tensor.matmul(out=acc[:], lhsT=Sw[:, c, :],
                                     rhs=G[:, c, :],
                                     start=(c == 0), stop=(c == ntot - 1))
                finish(sl, acc)

            for g, grp in enumerate(m3["groups"]):
                gviews = {}
                for h in (0, 1):
                    nidx, ioff = m3["calls"][(g, h)]
                    gapt = appool.tile([P, m3["nidxmax"]], u32, tag="APG")
                    nc.gpsimd.ap_gather(
                        out_ap=gapt[:, 0:nidx], in_ap=t3sb[:, h * APQ:(h + 1) * APQ],
                        idxs_ap=idx3full[:, ioff:ioff + nidx // 16],
                        channels=P, num_elems=APQ, d=1, num_idxs=nidx)
                    gviews[h] = gapt[:].bitcast(dt).rearrange(
                        "p (q two) -> p q two", two=2)
                for sl in grp:
                    process_ap_slot(sl, gviews)
                for sl in m3["dma_after"][g]:
                    process_dma_slot(sl)
    nc.compile()
    return nc


def _get_progs(key, meta, key3, m3):
    ck = (key, key3, L1_DT, SUP1_DT, SUP23_DT)
    if ck not in _PROG_CACHE:
        _PROG_CACHE[ck] = (_build_l1(), _build_l2(meta), _build_l3(m3))
    return _PROG_CACHE[ck]


# ------------------------------------------------------------------- kernel
def _run_spmd(nc, in_maps, tries=4):
    """run_bass_kernel_spmd with retries: the shared device pool occasionally
    needs a few minutes to recover a wedged worker."""
    import time
    for attempt in range(tries):
        try:
            return run_bass_kernel_spmd(nc, in_maps, core_ids=list(range(M)))
        except Exception:
            if attempt == tries - 1:
                raise
            time.sleep(90)


def kernel(x, W1, W2, W3, edge_weight, eps, edge_src, edge_dst):
    x = np.asarray(x, np.float32)
    W1 = np.asarray(W1, np.float32)
    W23 = np.concatenate([np.asarray(W2, np.float32),
                          np.asarray(W3, np.float32)], axis=1)
    eps = np.asarray(eps, np.float32)

    key, meta, eshards = _prep_edges(edge_src, edge_dst, edge_weight)
    key3, m3, eshards3 = _prep_edges3(edge_src, edge_dst, edge_weight)
    nc1, nc2, nc3 = _get_progs(key, meta, key3, m3)

    iota = np.broadcast_to(np.arange(P, dtype=np.float32)[None, :], (P, P))

    # ---- L1: support1 shards
    np1 = NPDT[L1_DT]
    in1 = []
    NPAD = NWIN * P
    for m in range(M):
        xs = np.zeros((NPAD, F_IN), np1)
        xs[:NSH] = x[m * NSH:(m + 1) * NSH].astype(np1)
        xLm = np.ascontiguousarray(
            xs.reshape(NPAD, KCH, P).transpose(2, 0, 1))   # [128, NPAD, KCH]
        in1.append({"xL": xLm, "W1": W1.astype(np1)})
    r1 = _run_spmd(nc1, in1)
    sup1 = np.concatenate([r1.results[m]["s1"][:NSH] for m in range(M)], axis=0)

    # window-slot permutation helpers (slot j on core m = window perm[j])
    def unslot(block, m, H):
        """[NWIN*P, H] slot-blocked -> [NSH, H] node-ordered for core m."""
        perm = eshards[m][2]
        out = np.empty((NSH, H), block.dtype)
        for j in range(NWIN):
            wj = int(perm[j])
            r = min(P, NSH - wj * P)
            out[wj * P:wj * P + r] = block[j * P:j * P + r]
        return out

    def toslot(arr, m):
        """[NSH, H] node-ordered -> [NWIN*P, H] slot-blocked for core m."""
        perm = eshards[m][2]
        out = np.zeros((NWIN * P, arr.shape[1]), arr.dtype)
        for j in range(NWIN):
            wj = int(perm[j])
            r = min(P, NSH - wj * P)
            out[j * P:j * P + r] = arr[wj * P:wj * P + r]
        return out

    # ---- L2: h1 + support23 shards
    np2 = NPDT[SUP1_DT]
    sup1 = sup1.astype(np2)
    in2 = [{"sup1A": sup1[:SPLIT], "sup1B": sup1[N - SPLIT:],
            "eidx": eshards[m][0], "emeta": eshards[m][1],
            "W23": W23, "iota": iota.astype(np2)}
           for m in range(M)]
    r2 = _run_spmd(nc2, in2)
    NPAIR = (NWIN + 1) // 2
    sup23_parts = []
    for m in range(M):
        pr = r2.results[m]["s23"].reshape(NPAIR, P, 2, H23)
        sl = np.empty((NWIN * P, H23), pr.dtype)
        for j in range(NWIN):
            sl[j * P:(j + 1) * P] = pr[j // 2, :, j % 2, :]
        sup23_parts.append(unslot(sl, m, H23))
    sup23 = np.concatenate(sup23_parts, axis=0)

    # ---- L3: mu, logvar, z shards
    np3 = NPDT[SUP23_DT]
    sup23 = sup23.astype(np3)
    t = sup23[:APN].reshape(APE, 2, H23)
    lo = t[:, 0, :].view(np.uint16).astype(np.uint32)
    hi = t[:, 1, :].view(np.uint16).astype(np.uint32)
    T3 = np.ascontiguousarray(((hi << 16) | lo).T)   # [128, APE] u32
    in3 = [{"sup23A": sup23[:SPLIT], "sup23B": sup23[N - SPLIT:],
            "eidx": eshards3[m][1], "eidx3": eshards3[m][0],
            "emeta": eshards3[m][2], "t3": T3,
            "iota": iota.astype(np3),
            "epss": np.ascontiguousarray(
                toslot(eps[m * NSH:(m + 1) * NSH], m)
                .reshape(NWIN, P, H2).transpose(1, 0, 2)
                .reshape(P, NWIN * H2)).astype(np.float16)}
           for m in range(M)]
    r3 = _run_spmd(nc3, in3)
    outs = [unslot(r3.results[m]["out3"], m, 3 * H2) for m in range(M)]
    full = np.concatenate(outs, axis=0)
    z, mu, logvar = full[:, 0:H2], full[:, H2:H23], full[:, H23:3 * H2]
    return (np.ascontiguousarray(z), np.ascontiguousarray(mu),
            np.ascontiguousarray(logvar))



# revision 30
# speedup vs baseline: 1.0949x; 1.0743x over previous
"""GCN-VAE encoder (2-layer GCN + reparameterize) on 8 Trainium2 NeuronCores.

Strategy (per the dst-sharding hint):
  - Nodes are sharded across the 8 cores (6250 rows each); edges are
    partitioned by dst and sorted by dst within each core.
  - Layer matmuls (x@W1, h1@[W2|W3]) run on the node shard each core owns.
  - The sparse aggregation out[d] = sum_{(s,d) in E} w * feat[s] is computed
    per 128-dst-node "window": edges are chunked into groups of 128, features
    feat[src] are fetched with dma_gather row gathers (two per window — the
    int16 gather index forces a <32768 split of the feature table), and the
    segment-sum is a PE matmul acc += S^T @ G with a one-hot matrix
    S[e, dst_local[e]] = w_e built on the vector engine.
  - Cross-core exchange of the full feature tables (support1, support23)
    happens through host round-trips between three SPMD launches (no
    on-device collectives needed):
      L1: support1_shard = x_shard @ W1
      L2: h1 = relu(spmm(support1_full));  support23_shard = h1 @ [W2|W3]
      L3: [mu|logvar] = relu(spmm(support23_full)); z = eps*exp(logvar)+mu
"""

import sys

for _p in ("/opt/trn_rl_repo", "/root/.axon_site/_ro/trn_rl_repo"):
    if _p not in sys.path:
        sys.path.append(_p)

import numpy as np
import ml_dtypes

import concourse.mybir as mybir
import concourse.tile as tile
from concourse import bacc
from concourse.bass_utils import run_bass_kernel_spmd
from concourse.masks import make_identity

# ---- problem constants (hardcoded per harness contract) ----
N, E, F_IN, H1, H2 = 50000, 1600000, 512, 256, 64
H23 = 2 * H2                      # concat(mu, logvar) feature width
M = 8                             # cores
NSH = N // M                      # nodes per core
P = 128                           # partitions / window size / edge chunk
NWIN = (NSH + P - 1) // P         # dst windows per core (49)
KCH = F_IN // P                   # k-chunks for layer-1 matmul (4)
SPLIT = 32768                     # int16 gather-index limit

f32 = mybir.dt.float32
f16 = mybir.dt.float16
i16 = mybir.dt.int16
u32 = mybir.dt.uint32

# L3 hybrid-lane parameters: edges of the first AP_SLOTS dst-slots whose src
# is below APN are gathered on-chip (gpsimd ap_gather from an SBUF-resident
# u32 node-pair-packed transposed sup23 table) instead of via DMA descriptors;
# this splits the gather load across the Pool engine and the DMA engines.
APN = 28672                       # nodes covered by the ap-lane pair table
APE = APN // 2                    # pair elements (u32 cols)
APQ = APE // 2                    # gather-view half (table-term balance)
AP_SLOTS = 24                     # slots using the ap-lane (rest pure DMA)
GRP = 6                           # ap slots per gather group

DT = {"f32": mybir.dt.float32, "f32r": mybir.dt.float32r,
      "f16": mybir.dt.float16, "bf16": mybir.dt.bfloat16}
NPDT = {"f32": np.float32, "f32r": np.float32,
        "f16": np.float16, "bf16": ml_dtypes.bfloat16}

# dtype ladder (accuracy/speed): "f32" exact, "f32r" single-pass PE fp32,
# "f16"/"bf16" half-width gather tables.
L1_DT = "f16"                     # x/W1 matmul operand dtype
SUP1_DT = "f16"                  # layer-1 feature table + S dtype
SUP23_DT = "f16"                  # layer-2/3 feature table + S dtype

_PROG_CACHE: dict = {}


# ---------------------------------------------------------------- host prep
def _wrap16(arr):
    """int16 gather-index layout: ordinal i -> [i%16, i//16], replicated to
    128 partitions (8 Q7 cores x 16)."""
    w = arr.astype(np.int16).reshape(-1, 16).T
    return np.tile(w, (8, 1))


def _prep_edges(edge_src, edge_dst, edge_weight):
    """Partition edges by dst shard, sort by (window, src-half), lay out
    per-window chunk tiles.

    Device arrays per core:
      EIDX [128, (totA+totB)*8] int16 — per window [A idxs | B idxs] wrapped
      EMETA [128, 2*totc] f32 — per window [dst_local (nw) | weight (nw)]
    Returns (key, meta, shards); meta holds the static chunk structure
    (identical across cores by construction)."""
    edge_src = np.asarray(edge_src).astype(np.int64)
    edge_dst = np.asarray(edge_dst).astype(np.int64)
    edge_weight = np.asarray(edge_weight).astype(np.float32)

    percore = []
    cntA = np.zeros((M, NWIN), np.int64)
    cntB = np.zeros((M, NWIN), np.int64)
    cntT = np.zeros((M, NWIN), np.int64)
    perms = []
    for m in range(M):
        sel = (edge_dst >= m * NSH) & (edge_dst < (m + 1) * NSH)
        d = edge_dst[sel] - m * NSH
        s = edge_src[sel]
        w = edge_weight[sel]
        win0 = d >> 7
        # slot-balance: rank windows by edge count so slot j holds each
        # core's j-th-largest window -> cross-core max padding shrinks
        wcnt = np.bincount(win0, minlength=NWIN)
        perm = np.argsort(-wcnt, kind="stable")           # slot -> window
        inv = np.empty(NWIN, np.int64)
        inv[perm] = np.arange(NWIN)                       # window -> slot
        perms.append(perm)
        win = inv[win0]                                   # slot index per edge
        klass = np.where(s < N - SPLIT, 0,
                         np.where(s < SPLIT, 1, 2))       # A-only/flex/B-only
        order = np.lexsort((klass, win))
        d, s, w, win, klass = d[order], s[order], w[order], win[order], klass[order]
        dloc = d - (perm[win] << 7)                       # dst_local in window
        for h, cnt in ((0, cntA), (2, cntB)):
            msk = klass == h
            cnt[m] = np.bincount(win[msk], minlength=NWIN)
        cntT[m] = np.bincount(win, minlength=NWIN)
        percore.append((dloc, s, w, win, klass))

    # chunk budget per slot: total rounded up, A sized to forced-A max,
    # flex edges fill A chunks to capacity before B
    tots = np.stack([np.bincount(pc[3], minlength=NWIN) for pc in percore])
    ncwT = np.maximum(2, -(-tots.max(axis=0) // P))
    ncwA = np.maximum(1, -(-cntA.max(axis=0) // P))
    while True:
        bad = (cntB.max(axis=0) > (ncwT - ncwA) * P)
        if not bad.any():
            break
        ncwT[bad] += 1
    ncwB = ncwT - ncwA
    ncw = ncwT
    offs = np.concatenate([[0], np.cumsum(ncw)])
    offsA = np.concatenate([[0], np.cumsum(ncwA)])
    offsB = np.concatenate([[0], np.cumsum(ncwB)])
    totc = int(offs[-1])
    totA, totB = int(offsA[-1]), int(offsB[-1])
    # exact (16-rounded) per-slot gather counts: cross-core max of edges
    # actually landing in each A/B half -> num_idxs below chunk-granular pad
    perA = np.minimum(cntT, ncwA[None, :] * P)
    exactA = (-(-perA.max(axis=0) // 16) * 16).astype(np.int64)
    exactB = (-(-(cntT - perA).max(axis=0) // 16) * 16).astype(np.int64)

    shards = []
    for m in range(M):
        dloc, s, w, win, klass = percore[m]
        DSTL = np.zeros((P, totc), np.float32)
        WGT = np.zeros((P, totc), np.float32)
        srcA = np.zeros(totA * P, np.int64)
        srcB = np.zeros(totB * P, np.int64)
        # edges are slot-major, class-ordered (A-only, flex, B-only): the
        # first capA go to half A (flex spills into A until its chunks are
        # full), the rest to half B
        wcnt = np.bincount(win, minlength=NWIN)
        starts = np.concatenate([[0], np.cumsum(wcnt)])[:-1]
        jall = np.arange(len(dloc)) - starts[win]          # rank within slot
        capA = ncwA[win] * P
        nAB = np.minimum(wcnt, ncwA * P)                   # A edges per slot
        toA = jall < capA
        for h, offsH, srcH, base_extra, sub in (
                (True, offsA, srcA, 0, 0),
                (False, offsB, srcB, None, SPLIT)):
            msk = toA if h else ~toA
            dh, sh, wh, winh = dloc[msk], s[msk], w[msk], win[msk]
            j = jall[msk] if h else jall[msk] - nAB[winh]
            rows = j % P
            base = offs[winh] + (0 if h else ncwA[winh])
            cols = base + j // P
            DSTL[rows, cols] = dh
            WGT[rows, cols] = wh
            srcH[offsH[winh] * P + j] = sh - (0 if h else N - SPLIT)
        IDXA = _wrap16(srcA)                               # [128, totA*8]
        IDXB = _wrap16(srcB)
        EIDX = np.zeros((P, (totA + totB) * 8), np.int16)
        EMETA = np.zeros((P, 2 * totc), np.float32)
        for t in range(NWIN):
            nA, nB = int(ncwA[t]), int(ncwB[t])
            co = (int(offsA[t]) + int(offsB[t])) * 8
            EIDX[:, co:co + nA * 8] = IDXA[:, offsA[t] * 8:(offsA[t] + nA) * 8]
            EIDX[:, co + nA * 8:co + (nA + nB) * 8] = \
                IDXB[:, offsB[t] * 8:(offsB[t] + nB) * 8]
            o = int(offs[t])
            EMETA[:, 2 * o:2 * o + (nA + nB)] = DSTL[:, o:o + nA + nB]
            EMETA[:, 2 * o + nA + nB:2 * (o + nA + nB)] = WGT[:, o:o + nA + nB]
        shards.append((EIDX, EMETA, perms[m]))

    key = (tuple(int(v) for v in ncwA) + tuple(int(v) for v in ncwB)
           + tuple(int(v) for v in exactA) + tuple(int(v) for v in exactB))
    meta = (tuple(int(v) for v in ncwA), tuple(int(v) for v in ncwB),
            offs, offsA, offsB, totc, totA, totB,
            tuple(int(v) for v in exactA), tuple(int(v) for v in exactB))
    return key, meta, shards


def _prep_edges3(edge_src, edge_dst, edge_weight):
    """L3 hybrid-lane edge layout.

    Slots 0..AP_SLOTS-1: edges with src < APN go to the ap-lane, laid out per
    (gather-group, table-half) call as 16-rounded (slot, parity) cell blocks
    of pair indices; src >= APN edges stay on the DMA lane (supB gather).
    Slots AP_SLOTS..: plain DMA A/B gathers (no flex balancing).
    Chunk metadata (dst_local | weight) is emitted slot-major into EMETA3."""
    edge_src = np.asarray(edge_src).astype(np.int64)
    edge_dst = np.asarray(edge_dst).astype(np.int64)
    edge_weight = np.asarray(edge_weight).astype(np.float32)

    # per-core per-slot edge lists, same slot permutation as _prep_edges
    cores = []
    for m in range(M):
        sel = (edge_dst >= m * NSH) & (edge_dst < (m + 1) * NSH)
        d = edge_dst[sel] - m * NSH
        s = edge_src[sel]
        w = edge_weight[sel]
        win0 = d >> 7
        wcnt = np.bincount(win0, minlength=NWIN)
        perm = np.argsort(-wcnt, kind="stable")
        inv = np.empty(NWIN, np.int64)
        inv[perm] = np.arange(NWIN)
        win = inv[win0]
        dloc = d - (perm[win] << 7)
        # cell id: ap slots: 0..3 = (half,parity), 4 = hi(DMA); dma slots: 5=A, 6=B
        is_ap = (win < AP_SLOTS) & (s < APN)
        cell = np.where(is_ap, ((s >> 1) >= APQ) * 2 + (s & 1),
                        np.where(win < AP_SLOTS, 4, np.where(s < SPLIT, 5, 6)))
        order = np.lexsort((cell, win))
        cores.append((dloc[order], s[order], w[order], win[order], cell[order]))

    # cross-core max counts per (slot, cell)
    cnt = np.zeros((M, NWIN, 7), np.int64)
    for m, (dloc, s, w, win, cell) in enumerate(cores):
        np.add.at(cnt[m], (win, cell), 1)
    cmax = cnt.max(axis=0)                       # [NWIN, 7]

    r16 = lambda v: -(-int(v) // 16) * 16
    nch_of = lambda v: max(0, -(-int(v) // P))

    groups = [list(range(g * GRP, min((g + 1) * GRP, AP_SLOTS)))
              for g in range(-(-AP_SLOTS // GRP))]
    dma_slots = list(range(AP_SLOTS, NWIN))
    nsp = len(groups)
    dma_after = [dma_slots[(len(dma_slots) * g) // nsp:
                           (len(dma_slots) * (g + 1)) // nsp] for g in range(nsp)]

    # ---- static layout
    apslots = {}
    calls = {}
    ic3 = 0
    for g, grp in enumerate(groups):
        for h in (0, 1):
            pos = 0
            for sl in grp:
                for par in (0, 1):
                    c = 2 * h + par
                    raw = cmax[sl, c]
                    nchc = nch_of(raw)
                    apslots.setdefault(sl, {"cells": [], "hi": None})
                    # blocks are 128-chunk aligned: the PE transpose's strided
                    # lhsT view silently corrupts at non-128 base offsets
                    apslots[sl]["cells"].append((h, par, pos, nchc))
                    pos += P * nchc
            nidx = max(pos, 16)
            calls[(g, h)] = (nidx, ic3)
            ic3 += nidx // 16

    icd = 0
    totc = 0
    for sl in range(NWIN):
        if sl < AP_SLOTS:
            nhi = nch_of(cmax[sl, 4])
            apslots[sl]["hi"] = (nhi, icd, r16(cmax[sl, 4]))
            ncells = sum(nc_ for (_, _, _, nc_) in apslots[sl]["cells"])
            apslots[sl]["ntot"] = ncells + nhi
            apslots[sl]["mc"] = totc
            totc += apslots[sl]["ntot"]
            icd += nhi * 8
        else:
            nA, nB = nch_of(cmax[sl, 5]), nch_of(cmax[sl, 6])
            apslots[sl] = {"dma": (nA, nB, icd, r16(cmax[sl, 5]), r16(cmax[sl, 6])),
                           "mc": totc, "ntot": nA + nB}
            totc += nA + nB
            icd += (nA + nB) * 8
    nidxmax = max(v[0] for v in calls.values())

    m3 = {"groups": groups, "dma_after": dma_after, "calls": calls,
          "apslots": apslots, "ic3": ic3, "icd": icd, "totc": totc,
          "nidxmax": nidxmax}

    # ---- per-core arrays
    shards3 = []
    for m, (dloc, s, w, win, cell) in enumerate(cores):
        EIDX3 = np.zeros((P, ic3), np.int16)
        EIDXD = np.zeros((P, icd), np.int16)
        EMETA3 = np.zeros((P, 2 * totc), np.float32)
        # per (slot, cell) edge slices (cells contiguous per lexsort)
        starts = {}
        k0 = 0
        keyarr = win * 8 + cell
        bounds = np.flatnonzero(np.diff(keyarr)) + 1
        segs = np.split(np.arange(len(win)), bounds)
        for seg in segs:
            if len(seg) == 0:
                continue
            starts[(int(win[seg[0]]), int(cell[seg[0]]))] = seg

        def fill_meta(sl, ci0, nchc, seg):
            ntot = apslots[sl]["ntot"]
            mc = apslots[sl]["mc"]
            if seg is None or nchc == 0:
                return
            j = np.arange(len(seg))
            rows, cols = j % P, ci0 + j // P
            EMETA3[rows, 2 * mc + cols] = dloc[seg]
            EMETA3[rows, 2 * mc + ntot + cols] = w[seg]

        for g, grp in enumerate(groups):
            for h in (0, 1):
                nidx, ioff = calls[(g, h)]
                buf = np.zeros(nidx, np.int64)
                for sl in grp:
                    ci = 0
                    for (hh, par, pos, nchc) in apslots[sl]["cells"]:
                        seg = starts.get((sl, 2 * hh + par))
                        if hh == h and seg is not None:
                            buf[pos:pos + len(seg)] = (s[seg] >> 1) - h * APQ
                        if hh == h:
                            pass
                        ci += nchc
                EIDX3[:, ioff:ioff + nidx // 16] = _wrap16(buf)
        # meta for ap cells + hi, and dma blocks
        for sl in range(NWIN):
            info = apslots[sl]
            if sl < AP_SLOTS:
                ci = 0
                for (hh, par, pos, nchc) in info["cells"]:
                    fill_meta(sl, ci, nchc, starts.get((sl, 2 * hh + par)))
                    ci += nchc
                nhi, doff, cnt16 = info["hi"]
                seg = starts.get((sl, 4))
                fill_meta(sl, ci, nhi, seg)
                if nhi:
                    buf = np.zeros(nhi * P, np.int64)
                    if seg is not None:
                        buf[:len(seg)] = s[seg] - (N - SPLIT)
                    EIDXD[:, doff:doff + nhi * 8] = _wrap16(buf)
            else:
                nA, nB, doff, cA16, cB16 = info["dma"]
                ci = 0
                for cc, nchc, sub, off2 in ((5, nA, 0, 0), (6, nB, N - SPLIT, nA * 8)):
                    seg = starts.get((sl, cc))
                    fill_meta(sl, ci, nchc, seg)
                    if nchc:
                        buf = np.zeros(nchc * P, np.int64)
                        if seg is not None:
                            buf[:len(seg)] = s[seg] - sub
                        EIDXD[:, doff + off2:doff + off2 + nchc * 8] = _wrap16(buf)
                    ci += nchc
        shards3.append((EIDX3, EIDXD, EMETA3))

    key3 = (ic3, icd, totc, nidxmax,
            tuple(apslots[sl]["ntot"] for sl in range(NWIN)),
            tuple(v[0] for v in calls.values()))
    return key3, m3, shards3


# ------------------------------------------------------------- bass builders
def _mk_nc():
    return bacc.Bacc("TRN2", target_bir_lowering=False, debug=False)


def _build_l1():
    """support1_shard[6250,256] = x_shard @ W1.

    xL is host-prepared as [128, NSH, KCH] (xL[p,n,k] = x[n, k*128+p]) so the
    whole shard loads SBUF-resident with four big contiguous DMAs; matmuls
    read stationary tiles straight out of it."""
    dt = DT[L1_DT]
    nc = _mk_nc()
    odt = DT[SUP1_DT]
    xL = nc.dram_tensor("xL", [P, NWIN * P, KCH], dt, kind="ExternalInput")
    W1 = nc.dram_tensor("W1", [F_IN, H1], dt, kind="ExternalInput")
    s1 = nc.dram_tensor("s1", [NWIN * P, H1], odt, kind="ExternalOutput")
    s1r = s1[:].rearrange("(t p) h -> p t h", p=P)          # [128, NWIN, H1]

    NSPLIT = 8
    NPAD = NWIN * P
    spans = [(NPAD * i // NSPLIT, NPAD * (i + 1) // NSPLIT) for i in range(NSPLIT)]
    with tile.TileContext(nc) as tc:
        with tc.tile_pool(name="const", bufs=1) as cpool, \
             tc.tile_pool(name="sbuf", bufs=6) as pool, \
             tc.tile_pool(name="psum", bufs=8, space="PSUM") as psum:
            w1c = cpool.tile([P, KCH, H1], dt)
            nc.sync.dma_start(out=w1c[:],
                              in_=W1[:].rearrange("(k p) n -> p k n", p=P))
            xfull = cpool.tile([P, NWIN * P, KCH], dt)
            for a, b in spans:
                nc.sync.dma_start(out=xfull[:, a:b, :], in_=xL[:, a:b, :])
            ofull = cpool.tile([P, NWIN, H1], odt)
            OSEC = 4
            sec = [(NWIN * i // OSEC, NWIN * (i + 1) // OSEC) for i in range(OSEC)]
            si = 0
            for t in range(NWIN):
                acc = psum.tile([P, H1], f32, space="PSUM", tag="acc")
                for k in range(KCH):
                    nc.tensor.matmul(
                        out=acc[:],
                        lhsT=xfull[:, t * P:(t + 1) * P, k],
                        rhs=w1c[:, k, :],
                        start=(k == 0), stop=(k == KCH - 1))
                nc.scalar.activation(out=ofull[:, t, :], in_=acc[:],
                                     func=mybir.ActivationFunctionType.Copy)
                if t + 1 == sec[si][1]:
                    a, b = sec[si]
                    nc.sync.dma_start(out=s1r[:, a:b, :], in_=ofull[:, a:b, :])
                    si += 1
    nc.compile()
    return nc


def _spmm_windows(nc, pool, psum, gpool, cpool, supA, supB, eidx, emeta,
                  iota_t, meta, H, dt, per_window_out):
    """Shared spmm structure: for each window slot produce PSUM [128, H]
    segment sum, then call per_window_out(win, rows, acc_psum). Edge index
    and metadata arrays are loaded SBUF-resident once up front."""
    ncwA, ncwB, offs, offsA, offsB, totc, totA, totB, exactA, exactB = meta
    metafull = cpool.tile([P, 2 * totc], f32)
    idxfull = cpool.tile([P, (totA + totB) * 8], i16)
    bnds = [0, NWIN // 4, NWIN // 2, (3 * NWIN) // 4, NWIN]
    for a, b in zip(bnds[:-1], bnds[1:]):
        mo, mo2 = 2 * int(offs[a]), 2 * int(offs[b])
        nc.sync.dma_start(out=metafull[:, mo:mo2], in_=emeta[:, mo:mo2])
        io = (int(offsA[a]) + int(offsB[a])) * 8
        io2 = (int(offsA[b]) + int(offsB[b])) * 8
        nc.sync.dma_start(out=idxfull[:, io:io2], in_=eidx[:, io:io2])
    for win in range(NWIN):
        rows = P
        nA, nB = ncwA[win], ncwB[win]
        nw = nA + nB
        off = int(offs[win])
        co = (int(offsA[win]) + int(offsB[win])) * 8
        meta_t = metafull[:, 2 * off:2 * (off + nw)]

        G = gpool.tile([P, nw, H], dt, tag="G")
        # exact counts shave chunk-granular padding; first pool rotation uses
        # the full padded count so no G rows are ever read uninitialized
        cA = nA * P if win < 4 else min(int(exactA[win]), nA * P)
        cB = nB * P if win < 4 else min(int(exactB[win]), nB * P)
        nc.gpsimd.dma_gather(
            out_ap=G[:, 0:nA, :], in_ap=supA[:],
            idxs_ap=idxfull[:, co:co + nA * 8],
            num_idxs=cA, num_idxs_reg=cA, elem_size=H,
            single_packet=False)
        nc.gpsimd.dma_gather(
            out_ap=G[:, nA:nw, :], in_ap=supB[:],
            idxs_ap=idxfull[:, co + nA * 8:co + nw * 8],
            num_idxs=cB, num_idxs_reg=cB, elem_size=H,
            single_packet=False)

        acc = psum.tile([P, H], f32, space="PSUM", tag="acc")
        S = pool.tile([P, nw, P], dt, tag="S")
        for c in range(nw):
            nc.vector.tensor_scalar(
                out=S[:, c, :], in0=iota_t[:],
                scalar1=meta_t[:, c:c + 1], scalar2=meta_t[:, nw + c:nw + c + 1],
                op0=mybir.AluOpType.is_equal, op1=mybir.AluOpType.mult)
            nc.tensor.matmul(
                out=acc[:],
                lhsT=S[:, c, :],
                rhs=G[:, c, :],
                start=(c == 0), stop=(c == nw - 1))
        per_window_out(win, rows, acc)


def _decl_spmm_inputs(nc, meta, H, dt, supname):
    totc, totA, totB = meta[5], meta[6], meta[7]
    supA = nc.dram_tensor(supname + "A", [SPLIT, H], dt, kind="ExternalInput")
    supB = nc.dram_tensor(supname + "B", [SPLIT, H], dt, kind="ExternalInput")
    eidx = nc.dram_tensor("eidx", [P, (totA + totB) * 8], i16, kind="ExternalInput")
    emeta = nc.dram_tensor("emeta", [P, 2 * totc], f32, kind="ExternalInput")
    iota_h = nc.dram_tensor("iota", [P, P], dt, kind="ExternalInput")
    return supA, supB, eidx, emeta, iota_h


def _build_l2(meta):
    """h1 = relu(spmm(support1)); support23_shard = h1 @ W23."""
    dt = DT[SUP1_DT]
    nc = _mk_nc()
    supA, supB, eidx, emeta, iota_h = _decl_spmm_inputs(nc, meta, H1, dt, "sup1")
    W23 = nc.dram_tensor("W23", [H1, H23], f32, kind="ExternalInput")
    odt = DT[SUP23_DT]
    NPAIR = (NWIN + 1) // 2
    s23 = nc.dram_tensor("s23", [NPAIR * P, 2 * H23], odt, kind="ExternalOutput")

    with tile.TileContext(nc) as tc:
        with tc.tile_pool(name="const", bufs=1) as cpool, \
             tc.tile_pool(name="sbuf", bufs=3) as pool, \
             tc.tile_pool(name="small", bufs=8) as spool, \
             tc.tile_pool(name="gpool", bufs=4) as gpool, \
             tc.tile_pool(name="psum", bufs=3, space="PSUM") as psum, \
             tc.tile_pool(name="psum2", bufs=2, space="PSUM") as psum2:
            iota_t = cpool.tile([P, P], dt)
            nc.sync.dma_start(out=iota_t[:], in_=iota_h[:])
            ident = cpool.tile([P, P], f32)
            make_identity(nc, ident[:])
            w23c = cpool.tile([P, H1 // P, H23], f32)
            nc.sync.dma_start(out=w23c[:],
                              in_=W23[:].rearrange("(k p) n -> p k n", p=P))

            opair_box = [None]

            def finish(win, rows, acc):
                h1 = pool.tile([P, H1], f32, tag="h1")
                nc.scalar.activation(out=h1[:], in_=acc[:],
                                     func=mybir.ActivationFunctionType.Relu)
                ps23 = psum2.tile([P, H23], f32, space="PSUM", tag="ps23")
                for fh in range(H1 // P):
                    tp = psum2.tile([P, P], f32, space="PSUM", tag="tp")
                    nc.tensor.transpose(out=tp[:], in_=h1[:, fh * P:(fh + 1) * P],
                                        identity=ident[:])
                    tps = pool.tile([P, P], f32, tag="tps")
                    nc.vector.tensor_copy(out=tps[:], in_=tp[:])
                    nc.tensor.matmul(
                        out=ps23[:],
                        lhsT=tps[:],
                        rhs=w23c[:, fh, :],
                        start=(fh == 0), stop=(fh == H1 // P - 1))
                if win % 2 == 0:
                    op_t = pool.tile([P, 2, H23], odt, tag="opair")
                    opair_box[0] = op_t
                opair = opair_box[0]
                nc.scalar.activation(out=opair[:, win % 2, :], in_=ps23[:],
                                     func=mybir.ActivationFunctionType.Copy)
                pb = win // 2
                if win % 2 == 1:
                    nc.sync.dma_start(out=s23[pb * P:(pb + 1) * P, :],
                                      in_=opair[:])
                elif win == NWIN - 1:
                    nc.sync.dma_start(out=s23[pb * P:(pb + 1) * P, 0:H23],
                                      in_=opair[:, 0, :])

            _spmm_windows(nc, spool, psum, gpool, cpool, supA, supB, eidx,
                          emeta, iota_t, meta, H1, dt, finish)
    nc.compile()
    return nc


def _build_l3(meta):
    """[mu|logvar] = relu(spmm(support23)); z = eps*exp(logvar)+mu."""
    dt = DT[SUP23_DT]
    nc = _mk_nc()
    supA, supB, eidx, emeta, iota_h = _decl_spmm_inputs(nc, meta, H23, dt, "sup23")
    epss = nc.dram_tensor("epss", [P, NWIN * H2], f16, kind="ExternalInput")
    out3 = nc.dram_tensor("out3", [NWIN * P, 3 * H2], f32, kind="ExternalOutput")

    with tile.TileContext(nc) as tc:
        with tc.tile_pool(name="const", bufs=1) as cpool, \
             tc.tile_pool(name="sbuf", bufs=3) as pool, \
             tc.tile_pool(name="small", bufs=8) as spool, \
             tc.tile_pool(name="gpool", bufs=4) as gpool, \
             tc.tile_pool(name="psum", bufs=6, space="PSUM") as psum:
            iota_t = cpool.tile([P, P], dt)
            nc.sync.dma_start(out=iota_t[:], in_=iota_h[:])
            epsfull = cpool.tile([P, NWIN, H2], f16)
            nc.sync.dma_start(out=epsfull[:], in_=epss[:])

            def finish(win, rows, acc):
                o = pool.tile([P, 3 * H2], f32, tag="o3")
                # o = [z | mu | logvar]
                nc.scalar.activation(out=o[:, H2:H23], in_=acc[:, 0:H2],
                                     func=mybir.ActivationFunctionType.Relu)
                nc.scalar.activation(out=o[:, H23:3 * H2], in_=acc[:, H2:H23],
                                     func=mybir.ActivationFunctionType.Relu)
                ex_t = pool.tile([P, H2], f32, tag="ex")
                nc.scalar.activation(out=ex_t[:], in_=o[:, H23:3 * H2],
                                     func=mybir.ActivationFunctionType.Exp)
                nc.vector.tensor_mul(out=o[:, 0:H2], in0=ex_t[:],
                                     in1=epsfull[:, win, :])
                nc.vector.tensor_add(out=o[:, 0:H2], in0=o[:, 0:H2],
                                     in1=o[:, H2:H23])
                nc.sync.dma_start(out=out3[win * P:(win + 1) * P, :], in_=o[:])

            _spmm_windows(nc, spool, psum, gpool, cpool, supA, supB, eidx,
                          emeta, iota_t, meta, H23, dt, finish)
    nc.compile()
    return nc


def _get_progs(key, meta, key3, m3):
    ck = (key, key3, L1_DT, SUP1_DT, SUP23_DT)
    if ck not in _PROG_CACHE:
        _PROG_CACHE[ck] = (_build_l1(), _build_l2(meta), _build_l3(meta))
    return _PROG_CACHE[ck]


# ------------------------------------------------------------------- kernel
def _run_spmd(nc, in_maps, tries=4):
    """run_bass_kernel_spmd with retries: the shared device pool occasionally
    needs a few minutes to recover a wedged worker."""
    import time
    for attempt in range(tries):
        try:
            return run_bass_kernel_spmd(nc, in_maps, core_ids=list(range(M)))
        except Exception:
            if attempt == tries - 1:
                raise
            time.sleep(90)


def kernel(x, W1, W2, W3, edge_weight, eps, edge_src, edge_dst):
    x = np.asarray(x, np.float32)
    W1 = np.asarray(W1, np.float32)
    W23 = np.concatenate([np.asarray(W2, np.float32),
                          np.asarray(W3, np.float32)], axis=1)
    eps = np.asarray(eps, np.float32)

    key, meta, eshards = _prep_edges(edge_src, edge_dst, edge_weight)
    key3, m3, eshards3 = _prep_edges3(edge_src, edge_dst, edge_weight)
    nc1, nc2, nc3 = _get_progs(key, meta, key3, m3)

    iota = np.broadcast_to(np.arange(P, dtype=np.float32)[None, :], (P, P))

    # ---- L1: support1 shards
    np1 = NPDT[L1_DT]
    in1 = []
    NPAD = NWIN * P
    for m in range(M):
        xs = np.zeros((NPAD, F_IN), np1)
        xs[:NSH] = x[m * NSH:(m + 1) * NSH].astype(np1)
        xLm = np.ascontiguousarray(
            xs.reshape(NPAD, KCH, P).transpose(2, 0, 1))   # [128, NPAD, KCH]
        in1.append({"xL": xLm, "W1": W1.astype(np1)})
    r1 = _run_spmd(nc1, in1)
    sup1 = np.concatenate([r1.results[m]["s1"][:NSH] for m in range(M)], axis=0)

    # window-slot permutation helpers (slot j on core m = window perm[j])
    def unslot(block, m, H):
        """[NWIN*P, H] slot-blocked -> [NSH, H] node-ordered for core m."""
        perm = eshards[m][2]
        out = np.empty((NSH, H), block.dtype)
        for j in range(NWIN):
            wj = int(perm[j])
            r = min(P, NSH - wj * P)
            out[wj * P:wj * P + r] = block[j * P:j * P + r]
        return out

    def toslot(arr, m):
        """[NSH, H] node-ordered -> [NWIN*P, H] slot-blocked for core m."""
        perm = eshards[m][2]
        out = np.zeros((NWIN * P, arr.shape[1]), arr.dtype)
        for j in range(NWIN):
            wj = int(perm[j])
            r = min(P, NSH - wj * P)
            out[j * P:j * P + r] = arr[wj * P:wj * P + r]
        return out

    # ---- L2: h1 + support23 shards
    np2 = NPDT[SUP1_DT]
    sup1 = sup1.astype(np2)
    in2 = [{"sup1A": sup1[:SPLIT], "sup1B": sup1[N - SPLIT:],
            "eidx": eshards[m][0], "emeta": eshards[m][1],
            "W23": W23, "iota": iota.astype(np2)}
           for m in range(M)]
    r2 = _run_spmd(nc2, in2)
    NPAIR = (NWIN + 1) // 2
    sup23_parts = []
    for m in range(M):
        pr = r2.results[m]["s23"].reshape(NPAIR, P, 2, H23)
        sl = np.empty((NWIN * P, H23), pr.dtype)
        for j in range(NWIN):
            sl[j * P:(j + 1) * P] = pr[j // 2, :, j % 2, :]
        sup23_parts.append(unslot(sl, m, H23))
    sup23 = np.concatenate(sup23_parts, axis=0)

    # ---- L3: mu, logvar, z shards
    np3 = NPDT[SUP23_DT]
    sup23 = sup23.astype(np3)
    in3 = [{"sup23A": sup23[:SPLIT], "sup23B": sup23[N - SPLIT:],
            "eidx": eshards[m][0], "emeta": eshards[m][1],
            "iota": iota.astype(np3),
            "epss": np.ascontiguousarray(
                toslot(eps[m * NSH:(m + 1) * NSH], m)
                .reshape(NWIN, P, H2).transpose(1, 0, 2)
                .reshape(P, NWIN * H2)).astype(np.float16)}
           for m in range(M)]
    r3 = _run_spmd(nc3, in3)
    outs = [unslot(r3.results[m]["out3"], m, 3 * H2) for m in range(M)]
    full = np.concatenate(outs, axis=0)
    z, mu, logvar = full[:, 0:H2], full[:, H2:H23], full[:, H23:3 * H2]
    return (np.ascontiguousarray(z), np.ascontiguousarray(mu),
            np.ascontiguousarray(logvar))



# revision 31
# speedup vs baseline: 1.1002x; 1.0049x over previous
"""GCN-VAE encoder (2-layer GCN + reparameterize) on 8 Trainium2 NeuronCores.

Strategy (per the dst-sharding hint):
  - Nodes are sharded across the 8 cores (6250 rows each); edges are
    partitioned by dst and sorted by dst within each core.
  - Layer matmuls (x@W1, h1@[W2|W3]) run on the node shard each core owns.
  - The sparse aggregation out[d] = sum_{(s,d) in E} w * feat[s] is computed
    per 128-dst-node "window": edges are chunked into groups of 128, features
    feat[src] are fetched with dma_gather row gathers (two per window — the
    int16 gather index forces a <32768 split of the feature table), and the
    segment-sum is a PE matmul acc += S^T @ G with a one-hot matrix
    S[e, dst_local[e]] = w_e built on the vector engine.
  - Cross-core exchange of the full feature tables (support1, support23)
    happens through host round-trips between three SPMD launches (no
    on-device collectives needed):
      L1: support1_shard = x_shard @ W1
      L2: h1 = relu(spmm(support1_full));  support23_shard = h1 @ [W2|W3]
      L3: [mu|logvar] = relu(spmm(support23_full)); z = eps*exp(logvar)+mu
"""

import sys

for _p in ("/opt/trn_rl_repo", "/root/.axon_site/_ro/trn_rl_repo"):
    if _p not in sys.path:
        sys.path.append(_p)

import numpy as np
import ml_dtypes

import concourse.mybir as mybir
import concourse.tile as tile
from concourse import bacc
from concourse.bass_utils import run_bass_kernel_spmd
from concourse.masks import make_identity

# ---- problem constants (hardcoded per harness contract) ----
N, E, F_IN, H1, H2 = 50000, 1600000, 512, 256, 64
H23 = 2 * H2                      # concat(mu, logvar) feature width
M = 8                             # cores
NSH = N // M                      # nodes per core
P = 128                           # partitions / window size / edge chunk
NWIN = (NSH + P - 1) // P         # dst windows per core (49)
KCH = F_IN // P                   # k-chunks for layer-1 matmul (4)
SPLIT = 32768                     # int16 gather-index limit

f32 = mybir.dt.float32
f16 = mybir.dt.float16
i16 = mybir.dt.int16
u32 = mybir.dt.uint32

# L3 hybrid-lane parameters: edges of the first AP_SLOTS dst-slots whose src
# is below APN are gathered on-chip (gpsimd ap_gather from an SBUF-resident
# u32 node-pair-packed transposed sup23 table) instead of via DMA descriptors;
# this splits the gather load across the Pool engine and the DMA engines.
APN = 28672                       # nodes covered by the ap-lane pair table
APE = APN // 2                    # pair elements (u32 cols)
APQ = APE // 2                    # gather-view half (table-term balance)
AP_SLOTS = 24                     # slots using the ap-lane (rest pure DMA)
GRP = 6                           # ap slots per gather group

DT = {"f32": mybir.dt.float32, "f32r": mybir.dt.float32r,
      "f16": mybir.dt.float16, "bf16": mybir.dt.bfloat16}
NPDT = {"f32": np.float32, "f32r": np.float32,
        "f16": np.float16, "bf16": ml_dtypes.bfloat16}

# dtype ladder (accuracy/speed): "f32" exact, "f32r" single-pass PE fp32,
# "f16"/"bf16" half-width gather tables.
L1_DT = "f16"                     # x/W1 matmul operand dtype
SUP1_DT = "f16"                  # layer-1 feature table + S dtype
SUP23_DT = "f16"                  # layer-2/3 feature table + S dtype

_PROG_CACHE: dict = {}


# ---------------------------------------------------------------- host prep
def _wrap16(arr):
    """int16 gather-index layout: ordinal i -> [i%16, i//16], replicated to
    128 partitions (8 Q7 cores x 16)."""
    w = arr.astype(np.int16).reshape(-1, 16).T
    return np.tile(w, (8, 1))


def _prep_edges(edge_src, edge_dst, edge_weight):
    """Partition edges by dst shard, sort by (window, src-half), lay out
    per-window chunk tiles.

    Device arrays per core:
      EIDX [128, (totA+totB)*8] int16 — per window [A idxs | B idxs] wrapped
      EMETA [128, 2*totc] f32 — per window [dst_local (nw) | weight (nw)]
    Returns (key, meta, shards); meta holds the static chunk structure
    (identical across cores by construction)."""
    edge_src = np.asarray(edge_src).astype(np.int64)
    edge_dst = np.asarray(edge_dst).astype(np.int64)
    edge_weight = np.asarray(edge_weight).astype(np.float32)

    percore = []
    cntA = np.zeros((M, NWIN), np.int64)
    cntB = np.zeros((M, NWIN), np.int64)
    cntT = np.zeros((M, NWIN), np.int64)
    perms = []
    for m in range(M):
        sel = (edge_dst >= m * NSH) & (edge_dst < (m + 1) * NSH)
        d = edge_dst[sel] - m * NSH
        s = edge_src[sel]
        w = edge_weight[sel]
        win0 = d >> 7
        # slot-balance: rank windows by edge count so slot j holds each
        # core's j-th-largest window -> cross-core max padding shrinks
        wcnt = np.bincount(win0, minlength=NWIN)
        perm = np.argsort(-wcnt, kind="stable")           # slot -> window
        inv = np.empty(NWIN, np.int64)
        inv[perm] = np.arange(NWIN)                       # window -> slot
        perms.append(perm)
        win = inv[win0]                                   # slot index per edge
        klass = np.where(s < N - SPLIT, 0,
                         np.where(s < SPLIT, 1, 2))       # A-only/flex/B-only
        order = np.lexsort((klass, win))
        d, s, w, win, klass = d[order], s[order], w[order], win[order], klass[order]
        dloc = d - (perm[win] << 7)                       # dst_local in window
        for h, cnt in ((0, cntA), (2, cntB)):
            msk = klass == h
            cnt[m] = np.bincount(win[msk], minlength=NWIN)
        cntT[m] = np.bincount(win, minlength=NWIN)
        percore.append((dloc, s, w, win, klass))

    # chunk budget per slot: total rounded up, A sized to forced-A max,
    # flex edges fill A chunks to capacity before B
    tots = np.stack([np.bincount(pc[3], minlength=NWIN) for pc in percore])
    ncwT = np.maximum(2, -(-tots.max(axis=0) // P))
    ncwA = np.maximum(1, -(-cntA.max(axis=0) // P))
    while True:
        bad = (cntB.max(axis=0) > (ncwT - ncwA) * P)
        if not bad.any():
            break
        ncwT[bad] += 1
    ncwB = ncwT - ncwA
    ncw = ncwT
    offs = np.concatenate([[0], np.cumsum(ncw)])
    offsA = np.concatenate([[0], np.cumsum(ncwA)])
    offsB = np.concatenate([[0], np.cumsum(ncwB)])
    totc = int(offs[-1])
    totA, totB = int(offsA[-1]), int(offsB[-1])
    # exact (16-rounded) per-slot gather counts: cross-core max of edges
    # actually landing in each A/B half -> num_idxs below chunk-granular pad
    perA = np.minimum(cntT, ncwA[None, :] * P)
    exactA = (-(-perA.max(axis=0) // 16) * 16).astype(np.int64)
    exactB = (-(-(cntT - perA).max(axis=0) // 16) * 16).astype(np.int64)

    shards = []
    for m in range(M):
        dloc, s, w, win, klass = percore[m]
        DSTL = np.zeros((P, totc), np.float32)
        WGT = np.zeros((P, totc), np.float32)
        srcA = np.zeros(totA * P, np.int64)
        srcB = np.zeros(totB * P, np.int64)
        # edges are slot-major, class-ordered (A-only, flex, B-only): the
        # first capA go to half A (flex spills into A until its chunks are
        # full), the rest to half B
        wcnt = np.bincount(win, minlength=NWIN)
        starts = np.concatenate([[0], np.cumsum(wcnt)])[:-1]
        jall = np.arange(len(dloc)) - starts[win]          # rank within slot
        capA = ncwA[win] * P
        nAB = np.minimum(wcnt, ncwA * P)                   # A edges per slot
        toA = jall < capA
        for h, offsH, srcH, base_extra, sub in (
                (True, offsA, srcA, 0, 0),
                (False, offsB, srcB, None, SPLIT)):
            msk = toA if h else ~toA
            dh, sh, wh, winh = dloc[msk], s[msk], w[msk], win[msk]
            j = jall[msk] if h else jall[msk] - nAB[winh]
            rows = j % P
            base = offs[winh] + (0 if h else ncwA[winh])
            cols = base + j // P
            DSTL[rows, cols] = dh
            WGT[rows, cols] = wh
            srcH[offsH[winh] * P + j] = sh - (0 if h else N - SPLIT)
        IDXA = _wrap16(srcA)                               # [128, totA*8]
        IDXB = _wrap16(srcB)
        EIDX = np.zeros((P, (totA + totB) * 8), np.int16)
        EMETA = np.zeros((P, 2 * totc), np.float32)
        for t in range(NWIN):
            nA, nB = int(ncwA[t]), int(ncwB[t])
            co = (int(offsA[t]) + int(offsB[t])) * 8
            EIDX[:, co:co + nA * 8] = IDXA[:, offsA[t] * 8:(offsA[t] + nA) * 8]
            EIDX[:, co + nA * 8:co + (nA + nB) * 8] = \
                IDXB[:, offsB[t] * 8:(offsB[t] + nB) * 8]
            o = int(offs[t])
            EMETA[:, 2 * o:2 * o + (nA + nB)] = DSTL[:, o:o + nA + nB]
            EMETA[:, 2 * o + nA + nB:2 * (o + nA + nB)] = WGT[:, o:o + nA + nB]
        shards.append((EIDX, EMETA, perms[m]))

    key = (tuple(int(v) for v in ncwA) + tuple(int(v) for v in ncwB)
           + tuple(int(v) for v in exactA) + tuple(int(v) for v in exactB))
    meta = (tuple(int(v) for v in ncwA), tuple(int(v) for v in ncwB),
            offs, offsA, offsB, totc, totA, totB,
            tuple(int(v) for v in exactA), tuple(int(v) for v in exactB))
    return key, meta, shards


def _prep_edges3(edge_src, edge_dst, edge_weight):
    """L3 hybrid-lane edge layout.

    Slots 0..AP_SLOTS-1: edges with src < APN go to the ap-lane, laid out per
    (gather-group, table-half) call as 16-rounded (slot, parity) cell blocks
    of pair indices; src >= APN edges stay on the DMA lane (supB gather).
    Slots AP_SLOTS..: plain DMA A/B gathers (no flex balancing).
    Chunk metadata (dst_local | weight) is emitted slot-major into EMETA3."""
    edge_src = np.asarray(edge_src).astype(np.int64)
    edge_dst = np.asarray(edge_dst).astype(np.int64)
    edge_weight = np.asarray(edge_weight).astype(np.float32)

    # per-core per-slot edge lists, same slot permutation as _prep_edges
    cores = []
    for m in range(M):
        sel = (edge_dst >= m * NSH) & (edge_dst < (m + 1) * NSH)
        d = edge_dst[sel] - m * NSH
        s = edge_src[sel]
        w = edge_weight[sel]
        win0 = d >> 7
        wcnt = np.bincount(win0, minlength=NWIN)
        perm = np.argsort(-wcnt, kind="stable")
        inv = np.empty(NWIN, np.int64)
        inv[perm] = np.arange(NWIN)
        win = inv[win0]
        dloc = d - (perm[win] << 7)
        # cell id: ap slots: 0..3 = (half,parity), 4 = hi(DMA); dma slots: 5=A, 6=B
        is_ap = (win < AP_SLOTS) & (s < APN)
        cell = np.where(is_ap, ((s >> 1) >= APQ) * 2 + (s & 1),
                        np.where(win < AP_SLOTS, 4, np.where(s < SPLIT, 5, 6)))
        order = np.lexsort((cell, win))
        cores.append((dloc[order], s[order], w[order], win[order], cell[order]))

    # cross-core max counts per (slot, cell)
    cnt = np.zeros((M, NWIN, 7), np.int64)
    for m, (dloc, s, w, win, cell) in enumerate(cores):
        np.add.at(cnt[m], (win, cell), 1)
    cmax = cnt.max(axis=0)                       # [NWIN, 7]

    r16 = lambda v: -(-int(v) // 16) * 16
    nch_of = lambda v: max(0, -(-int(v) // P))

    groups = [list(range(g * GRP, min((g + 1) * GRP, AP_SLOTS)))
              for g in range(-(-AP_SLOTS // GRP))]
    dma_slots = list(range(AP_SLOTS, NWIN))
    nsp = len(groups)
    dma_after = [dma_slots[(len(dma_slots) * g) // nsp:
                           (len(dma_slots) * (g + 1)) // nsp] for g in range(nsp)]

    # ---- static layout
    apslots = {}
    calls = {}
    ic3 = 0
    for g, grp in enumerate(groups):
        for h in (0, 1):
            pos = 0
            for sl in grp:
                for par in (0, 1):
                    c = 2 * h + par
                    raw = cmax[sl, c]
                    nchc = nch_of(raw)
                    apslots.setdefault(sl, {"cells": [], "hi": None})
                    # blocks are 128-chunk aligned: the PE transpose's strided
                    # lhsT view silently corrupts at non-128 base offsets
                    apslots[sl]["cells"].append((h, par, pos, nchc))
                    pos += P * nchc
            nidx = max(pos, 16)
            calls[(g, h)] = (nidx, ic3)
            ic3 += nidx // 16

    icd = 0
    totc = 0
    for sl in range(NWIN):
        if sl < AP_SLOTS:
            nhi = nch_of(cmax[sl, 4])
            apslots[sl]["hi"] = (nhi, icd, r16(cmax[sl, 4]))
            ncells = sum(nc_ for (_, _, _, nc_) in apslots[sl]["cells"])
            apslots[sl]["ntot"] = ncells + nhi
            apslots[sl]["mc"] = totc
            totc += apslots[sl]["ntot"]
            icd += nhi * 8
        else:
            nA, nB = nch_of(cmax[sl, 5]), nch_of(cmax[sl, 6])
            apslots[sl] = {"dma": (nA, nB, icd, r16(cmax[sl, 5]), r16(cmax[sl, 6])),
                           "mc": totc, "ntot": nA + nB}
            totc += nA + nB
            icd += (nA + nB) * 8
    nidxmax = max(v[0] for v in calls.values())

    m3 = {"groups": groups, "dma_after": dma_after, "calls": calls,
          "apslots": apslots, "ic3": ic3, "icd": icd, "totc": totc,
          "nidxmax": nidxmax}

    # ---- per-core arrays
    shards3 = []
    for m, (dloc, s, w, win, cell) in enumerate(cores):
        EIDX3 = np.zeros((P, ic3), np.int16)
        EIDXD = np.zeros((P, icd), np.int16)
        EMETA3 = np.zeros((P, 2 * totc), np.float32)
        # per (slot, cell) edge slices (cells contiguous per lexsort)
        starts = {}
        k0 = 0
        keyarr = win * 8 + cell
        bounds = np.flatnonzero(np.diff(keyarr)) + 1
        segs = np.split(np.arange(len(win)), bounds)
        for seg in segs:
            if len(seg) == 0:
                continue
            starts[(int(win[seg[0]]), int(cell[seg[0]]))] = seg

        def fill_meta(sl, ci0, nchc, seg):
            ntot = apslots[sl]["ntot"]
            mc = apslots[sl]["mc"]
            if seg is None or nchc == 0:
                return
            j = np.arange(len(seg))
            rows, cols = j % P, ci0 + j // P
            EMETA3[rows, 2 * mc + cols] = dloc[seg]
            EMETA3[rows, 2 * mc + ntot + cols] = w[seg]

        for g, grp in enumerate(groups):
            for h in (0, 1):
                nidx, ioff = calls[(g, h)]
                buf = np.zeros(nidx, np.int64)
                for sl in grp:
                    ci = 0
                    for (hh, par, pos, nchc) in apslots[sl]["cells"]:
                        seg = starts.get((sl, 2 * hh + par))
                        if hh == h and seg is not None:
                            buf[pos:pos + len(seg)] = (s[seg] >> 1) - h * APQ
                        if hh == h:
                            pass
                        ci += nchc
                EIDX3[:, ioff:ioff + nidx // 16] = _wrap16(buf)
        # meta for ap cells + hi, and dma blocks
        for sl in range(NWIN):
            info = apslots[sl]
            if sl < AP_SLOTS:
                ci = 0
                for (hh, par, pos, nchc) in info["cells"]:
                    fill_meta(sl, ci, nchc, starts.get((sl, 2 * hh + par)))
                    ci += nchc
                nhi, doff, cnt16 = info["hi"]
                seg = starts.get((sl, 4))
                fill_meta(sl, ci, nhi, seg)
                if nhi:
                    buf = np.zeros(nhi * P, np.int64)
                    if seg is not None:
                        buf[:len(seg)] = s[seg] - (N - SPLIT)
                    EIDXD[:, doff:doff + nhi * 8] = _wrap16(buf)
            else:
                nA, nB, doff, cA16, cB16 = info["dma"]
                ci = 0
                for cc, nchc, sub, off2 in ((5, nA, 0, 0), (6, nB, N - SPLIT, nA * 8)):
                    seg = starts.get((sl, cc))
                    fill_meta(sl, ci, nchc, seg)
                    if nchc:
                        buf = np.zeros(nchc * P, np.int64)
                        if seg is not None:
                            buf[:len(seg)] = s[seg] - sub
                        EIDXD[:, doff + off2:doff + off2 + nchc * 8] = _wrap16(buf)
                    ci += nchc
        shards3.append((EIDX3, EIDXD, EMETA3))

    key3 = (ic3, icd, totc, nidxmax,
            tuple(apslots[sl]["ntot"] for sl in range(NWIN)),
            tuple(v[0] for v in calls.values()))
    return key3, m3, shards3


# ------------------------------------------------------------- bass builders
def _mk_nc():
    return bacc.Bacc("TRN2", target_bir_lowering=False, debug=False)


def _build_l1():
    """support1_shard[6250,256] = x_shard @ W1.

    xL is host-prepared as [128, NSH, KCH] (xL[p,n,k] = x[n, k*128+p]) so the
    whole shard loads SBUF-resident with four big contiguous DMAs; matmuls
    read stationary tiles straight out of it."""
    dt = DT[L1_DT]
    nc = _mk_nc()
    odt = DT[SUP1_DT]
    xL = nc.dram_tensor("xL", [P, NWIN * P, KCH], dt, kind="ExternalInput")
    W1 = nc.dram_tensor("W1", [F_IN, H1], dt, kind="ExternalInput")
    s1 = nc.dram_tensor("s1", [NWIN * P, H1], odt, kind="ExternalOutput")
    s1r = s1[:].rearrange("(t p) h -> p t h", p=P)          # [128, NWIN, H1]

    NSPLIT = 8
    NPAD = NWIN * P
    spans = [(NPAD * i // NSPLIT, NPAD * (i + 1) // NSPLIT) for i in range(NSPLIT)]
    with tile.TileContext(nc) as tc:
        with tc.tile_pool(name="const", bufs=1) as cpool, \
             tc.tile_pool(name="sbuf", bufs=6) as pool, \
             tc.tile_pool(name="psum", bufs=8, space="PSUM") as psum:
            w1c = cpool.tile([P, KCH, H1], dt)
            nc.sync.dma_start(out=w1c[:],
                              in_=W1[:].rearrange("(k p) n -> p k n", p=P))
            xfull = cpool.tile([P, NWIN * P, KCH], dt)
            for a, b in spans:
                nc.sync.dma_start(out=xfull[:, a:b, :], in_=xL[:, a:b, :])
            ofull = cpool.tile([P, NWIN, H1], odt)
            OSEC = 4
            sec = [(NWIN * i // OSEC, NWIN * (i + 1) // OSEC) for i in range(OSEC)]
            si = 0
            for t in range(NWIN):
                acc = psum.tile([P, H1], f32, space="PSUM", tag="acc")
                for k in range(KCH):
                    nc.tensor.matmul(
                        out=acc[:],
                        lhsT=xfull[:, t * P:(t + 1) * P, k],
                        rhs=w1c[:, k, :],
                        start=(k == 0), stop=(k == KCH - 1))
                nc.scalar.activation(out=ofull[:, t, :], in_=acc[:],
                                     func=mybir.ActivationFunctionType.Copy)
                if t + 1 == sec[si][1]:
                    a, b = sec[si]
                    nc.sync.dma_start(out=s1r[:, a:b, :], in_=ofull[:, a:b, :])
                    si += 1
    nc.compile()
    return nc


def _spmm_windows(nc, pool, psum, gpool, cpool, supA, supB, eidx, emeta,
                  iota_t, meta, H, dt, per_window_out):
    """Shared spmm structure: for each window slot produce PSUM [128, H]
    segment sum, then call per_window_out(win, rows, acc_psum). Edge index
    and metadata arrays are loaded SBUF-resident once up front."""
    ncwA, ncwB, offs, offsA, offsB, totc, totA, totB, exactA, exactB = meta
    metafull = cpool.tile([P, 2 * totc], f32)
    idxfull = cpool.tile([P, (totA + totB) * 8], i16)
    bnds = [0, NWIN // 4, NWIN // 2, (3 * NWIN) // 4, NWIN]
    for a, b in zip(bnds[:-1], bnds[1:]):
        mo, mo2 = 2 * int(offs[a]), 2 * int(offs[b])
        nc.sync.dma_start(out=metafull[:, mo:mo2], in_=emeta[:, mo:mo2])
        io = (int(offsA[a]) + int(offsB[a])) * 8
        io2 = (int(offsA[b]) + int(offsB[b])) * 8
        nc.sync.dma_start(out=idxfull[:, io:io2], in_=eidx[:, io:io2])
    for win in range(NWIN):
        rows = P
        nA, nB = ncwA[win], ncwB[win]
        nw = nA + nB
        off = int(offs[win])
        co = (int(offsA[win]) + int(offsB[win])) * 8
        meta_t = metafull[:, 2 * off:2 * (off + nw)]

        G = gpool.tile([P, nw, H], dt, tag="G")
        # exact counts shave chunk-granular padding; first pool rotation uses
        # the full padded count so no G rows are ever read uninitialized
        cA = nA * P if win < 4 else min(int(exactA[win]), nA * P)
        cB = nB * P if win < 4 else min(int(exactB[win]), nB * P)
        nc.gpsimd.dma_gather(
            out_ap=G[:, 0:nA, :], in_ap=supA[:],
            idxs_ap=idxfull[:, co:co + nA * 8],
            num_idxs=cA, num_idxs_reg=cA, elem_size=H,
            single_packet=False)
        nc.gpsimd.dma_gather(
            out_ap=G[:, nA:nw, :], in_ap=supB[:],
            idxs_ap=idxfull[:, co + nA * 8:co + nw * 8],
            num_idxs=cB, num_idxs_reg=cB, elem_size=H,
            single_packet=False)

        acc = psum.tile([P, H], f32, space="PSUM", tag="acc")
        for c in range(nw):
            S = pool.tile([P, P], dt, tag="S")
            nc.vector.tensor_scalar(
                out=S[:], in0=iota_t[:],
                scalar1=meta_t[:, c:c + 1], scalar2=meta_t[:, nw + c:nw + c + 1],
                op0=mybir.AluOpType.is_equal, op1=mybir.AluOpType.mult)
            nc.tensor.matmul(
                out=acc[:],
                lhsT=S[:],
                rhs=G[:, c, :],
                start=(c == 0), stop=(c == nw - 1))
        per_window_out(win, rows, acc)


def _decl_spmm_inputs(nc, meta, H, dt, supname):
    totc, totA, totB = meta[5], meta[6], meta[7]
    supA = nc.dram_tensor(supname + "A", [SPLIT, H], dt, kind="ExternalInput")
    supB = nc.dram_tensor(supname + "B", [SPLIT, H], dt, kind="ExternalInput")
    eidx = nc.dram_tensor("eidx", [P, (totA + totB) * 8], i16, kind="ExternalInput")
    emeta = nc.dram_tensor("emeta", [P, 2 * totc], f32, kind="ExternalInput")
    iota_h = nc.dram_tensor("iota", [P, P], dt, kind="ExternalInput")
    return supA, supB, eidx, emeta, iota_h


def _build_l2(meta):
    """h1 = relu(spmm(support1)); support23_shard = h1 @ W23."""
    dt = DT[SUP1_DT]
    nc = _mk_nc()
    supA, supB, eidx, emeta, iota_h = _decl_spmm_inputs(nc, meta, H1, dt, "sup1")
    W23 = nc.dram_tensor("W23", [H1, H23], f32, kind="ExternalInput")
    odt = DT[SUP23_DT]
    NPAIR = (NWIN + 1) // 2
    s23 = nc.dram_tensor("s23", [NPAIR * P, 2 * H23], odt, kind="ExternalOutput")

    with tile.TileContext(nc) as tc:
        with tc.tile_pool(name="const", bufs=1) as cpool, \
             tc.tile_pool(name="sbuf", bufs=3) as pool, \
             tc.tile_pool(name="small", bufs=8) as spool, \
             tc.tile_pool(name="gpool", bufs=4) as gpool, \
             tc.tile_pool(name="psum", bufs=3, space="PSUM") as psum, \
             tc.tile_pool(name="psum2", bufs=2, space="PSUM") as psum2:
            iota_t = cpool.tile([P, P], dt)
            nc.sync.dma_start(out=iota_t[:], in_=iota_h[:])
            ident = cpool.tile([P, P], f32)
            make_identity(nc, ident[:])
            w23c = cpool.tile([P, H1 // P, H23], f32)
            nc.sync.dma_start(out=w23c[:],
                              in_=W23[:].rearrange("(k p) n -> p k n", p=P))

            opair_box = [None]

            def finish(win, rows, acc):
                h1 = pool.tile([P, H1], f32, tag="h1")
                nc.scalar.activation(out=h1[:], in_=acc[:],
                                     func=mybir.ActivationFunctionType.Relu)
                ps23 = psum2.tile([P, H23], f32, space="PSUM", tag="ps23")
                for fh in range(H1 // P):
                    tp = psum2.tile([P, P], f32, space="PSUM", tag="tp")
                    nc.tensor.transpose(out=tp[:], in_=h1[:, fh * P:(fh + 1) * P],
                                        identity=ident[:])
                    tps = pool.tile([P, P], f32, tag="tps")
                    nc.vector.tensor_copy(out=tps[:], in_=tp[:])
                    nc.tensor.matmul(
                        out=ps23[:],
                        lhsT=tps[:],
                        rhs=w23c[:, fh, :],
                        start=(fh == 0), stop=(fh == H1 // P - 1))
                if win % 2 == 0:
                    op_t = pool.tile([P, 2, H23], odt, tag="opair")
                    opair_box[0] = op_t
                opair = opair_box[0]
                nc.scalar.activation(out=opair[:, win % 2, :], in_=ps23[:],
                                     func=mybir.ActivationFunctionType.Copy)
                pb = win // 2
                if win % 2 == 1:
                    nc.sync.dma_start(out=s23[pb * P:(pb + 1) * P, :],
                                      in_=opair[:])
                elif win == NWIN - 1:
                    nc.sync.dma_start(out=s23[pb * P:(pb + 1) * P, 0:H23],
                                      in_=opair[:, 0, :])

            _spmm_windows(nc, spool, psum, gpool, cpool, supA, supB, eidx,
                          emeta, iota_t, meta, H1, dt, finish)
    nc.compile()
    return nc


def _build_l3(meta):
    """[mu|logvar] = relu(spmm(support23)); z = eps*exp(logvar)+mu."""
    dt = DT[SUP23_DT]
    nc = _mk_nc()
    supA, supB, eidx, emeta, iota_h = _decl_spmm_inputs(nc, meta, H23, dt, "sup23")
    epss = nc.dram_tensor("epss", [P, NWIN * H2], f16, kind="ExternalInput")
    out3 = nc.dram_tensor("out3", [NWIN * P, 3 * H2], f32, kind="ExternalOutput")

    with tile.TileContext(nc) as tc:
        with tc.tile_pool(name="const", bufs=1) as cpool, \
             tc.tile_pool(name="sbuf", bufs=3) as pool, \
             tc.tile_pool(name="small", bufs=8) as spool, \
             tc.tile_pool(name="gpool", bufs=4) as gpool, \
             tc.tile_pool(name="psum", bufs=6, space="PSUM") as psum:
            iota_t = cpool.tile([P, P], dt)
            nc.sync.dma_start(out=iota_t[:], in_=iota_h[:])
            epsfull = cpool.tile([P, NWIN, H2], f16)
            nc.sync.dma_start(out=epsfull[:], in_=epss[:])

            def finish(win, rows, acc):
                o = pool.tile([P, 3 * H2], f32, tag="o3")
                # o = [z | mu | logvar]
                nc.scalar.activation(out=o[:, H2:H23], in_=acc[:, 0:H2],
                                     func=mybir.ActivationFunctionType.Relu)
                nc.scalar.activation(out=o[:, H23:3 * H2], in_=acc[:, H2:H23],
                                     func=mybir.ActivationFunctionType.Relu)
                ex_t = pool.tile([P, H2], f32, tag="ex")
                nc.scalar.activation(out=ex_t[:], in_=o[:, H23:3 * H2],
                                     func=mybir.ActivationFunctionType.Exp)
                nc.vector.tensor_mul(out=o[:, 0:H2], in0=ex_t[:],
                                     in1=epsfull[:, win, :])
                nc.vector.tensor_add(out=o[:, 0:H2], in0=o[:, 0:H2],
                                     in1=o[:, H2:H23])
                nc.sync.dma_start(out=out3[win * P:(win + 1) * P, :], in_=o[:])

            _spmm_windows(nc, spool, psum, gpool, cpool, supA, supB, eidx,
                          emeta, iota_t, meta, H23, dt, finish)
    nc.compile()
    return nc


def _get_progs(key, meta, key3, m3):
    ck = (key, key3, L1_DT, SUP1_DT, SUP23_DT)
    if ck not in _PROG_CACHE:
        _PROG_CACHE[ck] = (_build_l1(), _build_l2(meta), _build_l3(meta))
    return _PROG_CACHE[ck]


# ------------------------------------------------------------------- kernel
def _run_spmd(nc, in_maps, tries=4):
    """run_bass_kernel_spmd with retries: the shared device pool occasionally
    needs a few minutes to recover a wedged worker."""
    import time
    for attempt in range(tries):
        try:
            return run_bass_kernel_spmd(nc, in_maps, core_ids=list(range(M)))
        except Exception:
            if attempt == tries - 1:
                raise
            time.sleep(90)


def kernel(x, W1, W2, W3, edge_weight, eps, edge_src, edge_dst):
    x = np.asarray(x, np.float32)
    W1 = np.asarray(W1, np.float32)
    W23 = np.concatenate([np.asarray(W2, np.float32),
                          np.asarray(W3, np.float32)], axis=1)
    eps = np.asarray(eps, np.float32)

    key, meta, eshards = _prep_edges(edge_src, edge_dst, edge_weight)
    key3, m3, eshards3 = _prep_edges3(edge_src, edge_dst, edge_weight)
    nc1, nc2, nc3 = _get_progs(key, meta, key3, m3)

    iota = np.broadcast_to(np.arange(P, dtype=np.float32)[None, :], (P, P))

    # ---- L1: support1 shards
    np1 = NPDT[L1_DT]
    in1 = []
    NPAD = NWIN * P
    for m in range(M):
        xs = np.zeros((NPAD, F_IN), np1)
        xs[:NSH] = x[m * NSH:(m + 1) * NSH].astype(np1)
        xLm = np.ascontiguousarray(
            xs.reshape(NPAD, KCH, P).transpose(2, 0, 1))   # [128, NPAD, KCH]
        in1.append({"xL": xLm, "W1": W1.astype(np1)})
    r1 = _run_spmd(nc1, in1)
    sup1 = np.concatenate([r1.results[m]["s1"][:NSH] for m in range(M)], axis=0)

    # window-slot permutation helpers (slot j on core m = window perm[j])
    def unslot(block, m, H):
        """[NWIN*P, H] slot-blocked -> [NSH, H] node-ordered for core m."""
        perm = eshards[m][2]
        out = np.empty((NSH, H), block.dtype)
        for j in range(NWIN):
            wj = int(perm[j])
            r = min(P, NSH - wj * P)
            out[wj * P:wj * P + r] = block[j * P:j * P + r]
        return out

    def toslot(arr, m):
        """[NSH, H] node-ordered -> [NWIN*P, H] slot-blocked for core m."""
        perm = eshards[m][2]
        out = np.zeros((NWIN * P, arr.shape[1]), arr.dtype)
        for j in range(NWIN):
            wj = int(perm[j])
            r = min(P, NSH - wj * P)
            out[j * P:j * P + r] = arr[wj * P:wj * P + r]
        return out

    # ---- L2: h1 + support23 shards
    np2 = NPDT[SUP1_DT]
    sup1 = sup1.astype(np2)
    in2 = [{"sup1A": sup1[:SPLIT], "sup1B": sup1[N - SPLIT:],
            "eidx": eshards[m][0], "emeta": eshards[m][1],
            "W23": W23, "iota": iota.astype(np2)}
           for m in range(M)]
    r2 = _run_spmd(nc2, in2)
    NPAIR = (NWIN + 1) // 2
    sup23_parts = []
    for m in range(M):
        pr = r2.results[m]["s23"].reshape(NPAIR, P, 2, H23)
        sl = np.empty((NWIN * P, H23), pr.dtype)
        for j in range(NWIN):
            sl[j * P:(j + 1) * P] = pr[j // 2, :, j % 2, :]
        sup23_parts.append(unslot(sl, m, H23))
    sup23 = np.concatenate(sup23_parts, axis=0)

    # ---- L3: mu, logvar, z shards
    np3 = NPDT[SUP23_DT]
    sup23 = sup23.astype(np3)
    in3 = [{"sup23A": sup23[:SPLIT], "sup23B": sup23[N - SPLIT:],
            "eidx": eshards[m][0], "emeta": eshards[m][1],
            "iota": iota.astype(np3),
            "epss": np.ascontiguousarray(
                toslot(eps[m * NSH:(m + 1) * NSH], m)
                .reshape(NWIN, P, H2).transpose(1, 0, 2)
                .reshape(P, NWIN * H2)).astype(np.float16)}
           for m in range(M)]
    r3 = _run_spmd(nc3, in3)
    outs = [unslot(r3.results[m]["out3"], m, 3 * H2) for m in range(M)]
    full = np.concatenate(outs, axis=0)
    z, mu, logvar = full[:, 0:H2], full[:, H2:H23], full[:, H23:3 * H2]
    return (np.ascontiguousarray(z), np.ascontiguousarray(mu),
            np.ascontiguousarray(logvar))

